# revision 55
# baseline (speedup 1.0000x reference)
"""Trainium2 Bass kernel for nn_AttributeOperator (MoE-style routing).

Computes out[b] = relu(attr_ops[attrs[b]] @ obj_emb[objs[b]]) for b in [0, B).

Strategy (expert-parallel): the dominant cost is streaming the attr_ops table
(N_ATTRS x D x D fp32 = 512 MB). Samples are grouped by attribute on the host,
groups are load-balanced across the 8 cores (snake deal by group size), and
each core streams only its own subset of operator matrices from HBM exactly
once, quantized on the host to fp8 e3m4 (TRN FP8_EXP3, 4 mantissa bits) with
a global x128 scale folded into the fp16 x vectors — 1 byte/elem halves the
HBM stream vs fp16 (rel err 1.39e-2 vs the f32 reference, under the 2e-2
gate; e4m3 fails at 3.2e-2). 63 matrices/core = 16.5 MB -> 46.2 us at the
358 GB/s per-core HBM cap; +xt/out traffic the HBM floor is ~48.2 us.

MODE="raw" (current, ~51.5 us/iter): hand-scheduled bass, no Tile
scheduling. Measured motivation: Tile attaches a sem update to every matmul
(they serialize at ~26 ns at the EVT_SEM register), inserts a full
drain+reset barrier between loop iterations (~4 us), and the PE+DMA streams
ended up only ~40% overlapped (66.5 us total vs 46 us DMA-alone). The raw
version:
  * credit/token semaphores whose wait targets live in per-engine registers
    (monotonic, iteration-invariant deltas) -> NO inter-iteration barrier;
    the 63-slot ops ring spans loop iterations and the act/out tail of
    iteration i overlaps the DMA stream of iteration i+1;
  * one sem inc per GROUP (then_inc on its last matmul; sound: PE
    completions are pc-ordered), one per-slot DMA-completion sem LANE
    (a single shared sem is UNSOUND with >1 DMA in flight: the 16 per-DMA
    increments come from 16 independent SDMA engines and interleave);
  * hybrid orientation: first 46 slots flip (A^T chunks stationary, fp8
    fast-weight-load, 16 matmuls/matrix, accumulating out^T in 4 PSUM
    banks), last NF=17 slots non-flip (x columns stationary, matrix rows
    moving, 4 matmuls/matrix into 4 dedicated PSUM banks cycled with
    eager per-slot DVE relu). Measured: a concurrent SDMA stream is slowed
    ~9 us by the flip-only PE instruction stream (1008 instr); the hybrid
    (756+68 instr, PE busy still < DMA) cuts that to ~3 us. All-non-flip
    is PE-bound (54 us) and slower. NB tile_position col-packing for the
    nf PSUM crashes the exec unit (NRT 101) — use whole banks only.
Sems are zeroed at kernel end so NEFF re-executions start clean; sem lanes
use nb=63 (= ng) so every lane has exactly one DMA/iteration and all lanes
share one wait-target register.

Previous Tile-based implementation (MODE="tile", ~64.6-66.5 us) is kept
for A/B; its ablations: fp16 streaming 108 us, fp8 DMA alone 47 us, PE
alone 39 us. The SPMD program is identical on all 8 cores; only per-core
tensors differ. Slot s has a fixed column capacity maxc[s] = max over
cores of that rank's group size, so the one program fits every core's
routing. Timing method (test.py): paired hardware-loop wall-delta,
(wall(R=2001) - wall(R=201)) / 1800.
"""

import numpy as np
import ml_dtypes

import concourse.tile as tile
from concourse import bacc, mybir
from concourse.bass_utils import run_bass_kernel_spmd

N_CORES = 8
D = 512               # embedding dim (hardcoded per problem spec)
QCH = D // 128        # contraction chunks of 128 partitions
# attr_ops stream is fp8 e3m4 (TRN FP8_EXP3): normals cover [0.25, 15.5], so
# scale A up by 128 (|A|max ~0.11 -> ~13.9) and fold 1/128 into x on the host.
A_SCALE = 128.0
E3M4 = ml_dtypes.float8_e3m4

# test.py hooks (ignored by the grading harness)
LAST_RESULTS = None   # BassKernelResults of the most recent run
TRACE = False
TRACE_CORES = None

PAIR = 1
# Slot column offsets aligned to 4 -> every matmul's f32 PSUM write starts
# 16B-cacheline-aligned and every fp16 xt read 8B-aligned (PSUM lines are
# 8B, SBUF lines 16B; misaligned partial-line PSUM writes measurably slow
# the matmul stream: align=4 beat align=1 by ~5us/iter in-process).
ALIGN = 4
_NC_CACHE = {}


def _build_nc(maxc, offs, ncol, ops_bufs=8, pair=1, sync_frac=(1, 1), reps=1,
              out_engine="scalar", staggered=False, relu_engine="scalar",
              xt_engine="scalar", ops_dt="f8e3", out_dt="f16",
              do_ops_dma=True, do_mm=True, do_act=True, do_out=True):
    """Build + compile the SPMD program.

    maxc[s]: column capacity of slot s; offs[s]: column offset of slot s;
    ncol: total columns (= offs[-1] + maxc[-1]).
    pair: matrices loaded per ops DMA (amortizes per-DMA fixed costs).
    sync_frac: (a, b) -> a of every b ops DMAs issue on sync, rest on scalar.
    reps: hardware-loop repetitions of the whole kernel (for timing).
    staggered: staggered-reset loop back-edge — wedges this device, keep False.
    """
    nm = len(maxc)
    nmp = -(-nm // pair) * pair  # nm rounded up to a multiple of pair
    ng = nmp // pair
    mdt = {"f8e3": mybir.dt.float8e3, "f8e4": mybir.dt.float8e4,
           "f16": mybir.dt.float16}[ops_dt]
    odt = {"f16": mybir.dt.float16, "f32": mybir.dt.float32}[out_dt]
    nc = bacc.Bacc("TRN2", target_bir_lowering=False, debug=False,
                   num_devices=N_CORES)
    # per-group layout [p, t, q, i]: each partition's data is one contiguous
    # pair*QCH*D-element run -> one big DMA descriptor per partition
    ops_dram = nc.dram_tensor("ops_t", [ng, 128, pair * QCH * D],
                              mdt, kind="ExternalInput").ap()
    xt_dram = nc.dram_tensor("xt", [128, QCH * ncol], mybir.dt.float16,
                             kind="ExternalInput").ap()
    out_dram = nc.dram_tensor("out", [ncol, D], odt,
                              kind="ExternalOutput").ap()

    with tile.TileContext(nc) as tc:
        with (
            tc.tile_pool(name="xt", bufs=1) as xt_pool,
            tc.tile_pool(name="ops", bufs=ops_bufs) as ops_pool,
            tc.tile_pool(name="ps", bufs=8, space="PSUM") as ps_pool,
            tc.tile_pool(name="o", bufs=4) as o_pool,
        ):
            def body():
                xt_sb = xt_pool.tile([128, QCH * ncol], mybir.dt.float16)
                getattr(nc, xt_engine).dma_start(xt_sb[:], xt_dram[:])
                if not do_ops_dma:
                    m0 = ops_pool.tile([128, pair * QCH * D], mdt, tag="m")
                    nc.sync.dma_start(m0[:], ops_dram[0])

                for g in range(ng):
                    if do_ops_dma:
                        m = ops_pool.tile([128, pair * QCH * D], mdt, tag="m")
                        issuer = nc.sync if g % sync_frac[1] < sync_frac[0] \
                            else nc.scalar
                        issuer.dma_start(m[:], ops_dram[g])
                    else:
                        m = m0
                    for t in range(pair):
                        s = g * pair + t
                        if s >= nm:
                            break
                        cw = maxc[s]
                        if not do_mm:
                            continue
                        ps = ps_pool.tile([cw, D], mybir.dt.float32, tag="ps")
                        for q in range(QCH):
                            lhsT = xt_sb[:, q * ncol + offs[s]:
                                         q * ncol + offs[s] + cw]
                            rhs = m[:, (t * QCH + q) * D:
                                    (t * QCH + q + 1) * D]
                            nc.tensor.matmul(ps[:], lhsT, rhs,
                                             start=(q == 0),
                                             stop=(q == QCH - 1))
                        if not do_act:
                            continue
                        o = o_pool.tile([cw, D], odt, tag="o")
                        if relu_engine == "vector":
                            nc.vector.tensor_scalar_max(o[:], ps[:], 0.0)
                        else:
                            nc.scalar.activation(
                                o[:], ps[:], mybir.ActivationFunctionType.Relu)
                        if not do_out:
                            continue
                        out_eng = getattr(nc, out_engine)
                        out_eng.dma_start(
                            out_dram[offs[s]:offs[s] + cw, :], o[:])

            if reps == 1:
                body()
            else:
                with tc.For_i(0, reps, 1,
                              hint_engines=(mybir.EngineType.PE,),
                              staggered_reset=staggered):
                    body()

    nc.compile()
    return nc


def _build_nc_flip(maxc, offs, ncol, ops_bufs=8, pair=1, sync_frac=(1, 1),
                   reps=1, out_engine="scalar", staggered=False,
                   relu_engine="vector", xt_engine="scalar", ops_dt="f8e3",
                   out_dt="f16", do_ops_dma=True, do_mm=True, do_act=True,
                   do_out=True, mm_src="real", mm_every=1, dma_split=False,
                   ops_engine=None, act_split=2, xt_split=True,
                   out_per_qi=True, mm_order="qi", mm_split=1,
                   ops_frac=1, prog_out=0):
    """Flipped orientation: A chunks are the stationary operand (fp8 weights
    -> fast weight load), x columns stream as the moving operand.

    Per slot s (one operator matrix A), for each output chunk qi and
    contraction chunk qj: ldweights(A^T[qj,qi] 128x128) + matmul over the
    slot's cw x-columns, accumulating out^T[qi*128:+128, cols(s)] in a PSUM
    tile [128, ncol] shared by all slots. One ReLU per qi over the full
    [128, ncol] bank, one contiguous output DMA of out^T.
    """
    nm = len(maxc)
    nmp = -(-nm // pair) * pair
    ng = nmp // pair
    mdt = {"f8e3": mybir.dt.float8e3, "f8e4": mybir.dt.float8e4,
           "f16": mybir.dt.float16}[ops_dt]
    odt = {"f16": mybir.dt.float16, "f32": mybir.dt.float32}[out_dt]
    nc = bacc.Bacc("TRN2", target_bir_lowering=False, debug=False,
                   num_devices=N_CORES)
    # ops_t[g, p, ((t*QCH+qj)*QCH+qi)*128 + i] = s*A_s[qi*128+i, qj*128+p]
    # ops_frac>1: timing-probe mode — stream only 1/ops_frac of the bytes
    # (results are garbage; used to measure DMA-vs-PE scaling).
    gsz = pair * QCH * QCH * 128 // ops_frac
    ops_dram = nc.dram_tensor("ops_t", [ng, 128, gsz],
                              mdt, kind="ExternalInput").ap()
    xt_dram = nc.dram_tensor("xt", [128, QCH * ncol], mybir.dt.float16,
                             kind="ExternalInput").ap()
    # out^T: out_dram[p, qi*ncol + c] = out[c, qi*128+p]
    out_dram = nc.dram_tensor("out", [128, QCH * ncol], odt,
                              kind="ExternalOutput").ap()

    with tile.TileContext(nc) as tc:
        with (
            tc.tile_pool(name="xt", bufs=1) as xt_pool,
            tc.tile_pool(name="ops", bufs=ops_bufs) as ops_pool,
            tc.tile_pool(name="m0p", bufs=1) as m0_pool,
            tc.tile_pool(name="ps", bufs=8, space="PSUM") as ps_pool,
            tc.tile_pool(name="o", bufs=2) as o_pool,
        ):
            def body():
                xt_sb = xt_pool.tile([128, QCH * ncol], mybir.dt.float16)
                if xt_split:
                    for qj in range(QCH):
                        getattr(nc, xt_engine).dma_start(
                            xt_sb[:, qj * ncol:(qj + 1) * ncol],
                            xt_dram[:, qj * ncol:(qj + 1) * ncol])
                else:
                    getattr(nc, xt_engine).dma_start(xt_sb[:], xt_dram[:])
                ps = [ps_pool.tile([128, ncol], mybir.dt.float32, tag="ps",
                                   name=f"ps{qi}")
                      for qi in range(QCH)]
                if not do_ops_dma or mm_src == "m0":
                    m0 = m0_pool.tile([128, gsz], mdt, tag="m0", bufs=1)
                    nc.sync.dma_start(m0[:], ops_dram[0])

                # progressive act/out: emit ReLU + out DMA for a column span
                # once the last slot covering it has been multiplied, instead
                # of serially after the whole stream.
                span_end = {}           # slot s -> (c0, c1) to flush after s
                o_prog = None
                if prog_out and do_act:
                    o_prog = o_pool.tile([128, QCH * ncol], odt, tag="o",
                                         bufs=1)
                    bounds = [nm * (j + 1) // prog_out - 1
                              for j in range(prog_out)]
                    c_prev = 0
                    for s_e in bounds:
                        c_hi = offs[s_e] + maxc[s_e]
                        span_end[s_e] = (c_prev, c_hi)
                        c_prev = c_hi

                def flush_span(c0, c1):
                    for qi in range(QCH):
                        dst = o_prog[:, qi * ncol + c0:qi * ncol + c1]
                        src = ps[qi][:, c0:c1]
                        if relu_engine == "vector":
                            nc.vector.tensor_scalar_max(dst, src, 0.0)
                        else:
                            nc.scalar.activation(
                                dst, src, mybir.ActivationFunctionType.Relu)
                        if do_out:
                            getattr(nc, out_engine).dma_start(
                                out_dram[:, qi * ncol + c0:qi * ncol + c1],
                                dst)

                for g in range(ng):
                    if do_ops_dma:
                        m = ops_pool.tile([128, gsz], mdt, tag="m")
                        if dma_split:
                            h = gsz // 2
                            nc.sync.dma_start(m[:, :h], ops_dram[g][:, :h])
                            nc.scalar.dma_start(m[:, h:], ops_dram[g][:, h:])
                        elif ops_engine is not None:
                            getattr(nc, ops_engine).dma_start(
                                m[:], ops_dram[g])
                        else:
                            issuer = nc.sync \
                                if g % sync_frac[1] < sync_frac[0] \
                                else nc.scalar
                            issuer.dma_start(m[:], ops_dram[g])
                        if mm_src == "m0":
                            m = m0
                    else:
                        m = m0
                    for t in range(pair):
                        s = g * pair + t
                        if s >= nm:
                            break
                        cw = maxc[s]
                        if not do_mm or s % mm_every:
                            continue
                        order = [(qi, qj) for qi in range(QCH)
                                 for qj in range(QCH)] \
                            if mm_order == "qi" else \
                            [(qi, qj) for qj in range(QCH)
                             for qi in range(QCH)]
                        for qi, qj in order:
                            ck = (((t * QCH + qj) * QCH + qi)
                                  % (gsz // 128)) * 128
                            lhsT = m[:, ck:ck + 128]
                            for h in range(mm_split):
                                a0 = offs[s] + cw * h // mm_split
                                a1 = offs[s] + cw * (h + 1) // mm_split
                                if a1 == a0:
                                    continue
                                rhs = xt_sb[:, qj * ncol + a0:
                                            qj * ncol + a1]
                                nc.tensor.matmul(
                                    ps[qi][:, a0:a1],
                                    lhsT, rhs, start=(qj == 0),
                                    stop=(qj == QCH - 1))
                if not do_act:
                    return
                o = o_pool.tile([128, QCH * ncol], odt, tag="o")
                for qi in range(QCH):
                    for h in range(act_split):
                        c0 = ncol * h // act_split
                        c1 = ncol * (h + 1) // act_split
                        dst = o[:, qi * ncol + c0:qi * ncol + c1]
                        src = ps[qi][:, c0:c1]
                        if relu_engine == "vector":
                            nc.vector.tensor_scalar_max(dst, src, 0.0)
                        else:
                            nc.scalar.activation(
                                dst, src, mybir.ActivationFunctionType.Relu)
                    if do_out and out_per_qi:
                        getattr(nc, out_engine).dma_start(
                            out_dram[:, qi * ncol:(qi + 1) * ncol],
                            o[:, qi * ncol:(qi + 1) * ncol])
                if do_out and not out_per_qi:
                    getattr(nc, out_engine).dma_start(out_dram[:], o[:])

            if reps == 1:
                body()
            else:
                with tc.For_i(0, reps, 1,
                              hint_engines=(mybir.EngineType.PE,),
                              staggered_reset=staggered):
                    body()

    nc.compile()
    return nc


NF = 17   # tail slots computed in non-flip orientation (hybrid)


def _build_raw(maxc, offs, ncol, reps=1, nb=63, do_ops_dma=True, do_mm=True,
               do_act=True, do_out=True, xt_chunks=1, use_regs=True,
               do_clear=False, ops_engine="sync", slot_stride=1,
               nf_probe=0, nf=None, pair=1):
    """Raw-bass (no Tile scheduling) flip-orientation kernel.

    Motivation (measured): Tile attaches a semaphore update to every matmul
    (1008/iter) and a full engine drain+reset between loop iterations; the
    per-instruction sem updates serialize at ~26ns each at the EVT_SEM
    register and the PE+DMA streams end up only ~40% overlapped (base 66.5us
    vs 46us DMA-alone line rate). This version hand-manages semaphores:
    one inc per ops-DMA (hardware, +16), one inc per GROUP on PE (last
    matmul's then_inc, sound because PE completions are pc-ordered), and a
    credit scheme whose wait targets are carried in per-engine registers so
    the hardware loop needs NO inter-iteration barrier: the ops-DMA ring
    spans iteration boundaries and the act/out tail of iteration i overlaps
    the DMA stream of iteration i+1.

    Engine program (per iteration):
      sync  : ng x [credit wait on pe_sem; dma_start ops -> ring (+16 dma_sem)]
      tensor: wait xt_sem; wait act_sem (PSUM free);
              ng x [wait dma_sem; 16 matmuls; last +1 pe_sem]
      scalar: wait pe_sem (xt free); dma xt (+16 xt_sem);
              4 x [wait act_sem; dma out qi (+16 out_sem)]
      vector: wait pe_sem (all groups); wait out_sem (o free);
              8 x relu chunk (+1 act_sem each)
    """
    nm = len(maxc)
    ng = nm
    if nf is None:
        nf = 0 if nf_probe else NF
    nf = min(nf, ng)
    assert all(maxc[s] <= 32 for s in range(ng - nf, ng))
    c_flip = offs[ng - nf] if nf else ncol      # first non-flip column
    noffs = [0]                                  # out_nf row offsets
    for s in range(ng - nf, ng):
        noffs.append(noffs[-1] + maxc[s])
    n_out = 4 + nf                               # out DMAs per iteration
    n_act = 8 + nf                               # act_sem incs per iteration

    nc = bacc.Bacc("TRN2", target_bir_lowering=False, debug=False,
                   num_devices=N_CORES)
    gsz = QCH * QCH * 128
    # pair>1: `pair` matrices per DMA (bigger per-partition runs amortize
    # SDMA packet overhead); the last DMA is truncated so padding matrices
    # are never transferred. Forces whole-iteration ring (nb = ng).
    ng_dma = -(-ng // pair)
    gszp = pair * gsz
    if pair > 1:
        nb = ng
    ops_dram = nc.dram_tensor("ops_t", [ng_dma, 128, gszp],
                              mybir.dt.float8e3,
                              kind="ExternalInput").ap()
    xt_dram = nc.dram_tensor("xt", [128, QCH * ncol], mybir.dt.float16,
                             kind="ExternalInput").ap()
    out_dram = nc.dram_tensor("out", [128, QCH * ncol], mybir.dt.float16,
                              kind="ExternalOutput").ap()
    out_nf_dram = nc.dram_tensor(
        "out_nf", [max(1, noffs[-1]), D], mybir.dt.float16,
        kind="ExternalOutput").ap()

    xt_sb = nc.alloc_sbuf_tensor("xt_sb", [128, QCH * ncol],
                                 mybir.dt.float16).ap()
    o_sb = nc.alloc_sbuf_tensor("o_sb", [128, QCH * ncol],
                                mybir.dt.float16).ap()
    o_nf = nc.alloc_sbuf_tensor("o_nf", [128, 4 * D], mybir.dt.float16).ap()
    mring = nc.alloc_sbuf_tensor("mring", [128, ng_dma * gszp],
                                 mybir.dt.float8e3).ap()
    ps = [nc.alloc_psum_tensor(f"ps{qi}", [128, ncol], mybir.dt.float32).ap()
          for qi in range(QCH)]
    # 4 PSUM banks for the non-flip tail, cycled with reuse distance 4
    # (no tile_position partition packing: that path crashes the exec unit)
    ps_nf = [nc.alloc_psum_tensor(f"ps_nf{b}", [128, D],
                                  mybir.dt.float32).ap()
             for b in range(4)] if (nf or nf_probe) else None

    # dma completion sems: one LANE per ring slot. A single shared sem is
    # unsound with >1 DMA in flight: the 16 per-DMA increments come from 16
    # independent SDMA engines, so increments of DMA k+1 can stand in for
    # laggards of DMA k and a threshold wait passes before k fully lands
    # (observed: corruption starting exactly where PE catches the stream).
    # With lane=slot and ring credits bounding in-flight <= nb, each lane
    # has at most one DMA outstanding -> its count is unambiguous.
    # nb must divide ng so lane assignment is iteration-invariant.
    assert ng % nb == 0, (ng, nb)
    n_lanes = ng_dma if pair > 1 else nb
    dma_sems = [nc.alloc_semaphore(f"dma_sem{l}") for l in range(n_lanes)]
    xt_sem = nc.alloc_semaphore("xt_sem")
    pe_sem = nc.alloc_semaphore("pe_sem")
    act_sem = nc.alloc_semaphore("act_sem")
    out_sem = nc.alloc_semaphore("out_sem")
    sems = dma_sems + [xt_sem, pe_sem, act_sem, out_sem]

    # NB: do NOT clear sems at kernel start — both a gpsimd sem_clear +
    # _nrt_pseudo_barrier and a sem_clear + all_engine_barrier preamble
    # measurably RACE the engine streams here (verified: rel err jumps to
    # ~0.3-0.8). Instead sems are zeroed at kernel END (below), and the
    # program assumes zeroed sems at entry.
    if do_clear:
        for s in sems:
            nc.gpsimd.sem_clear(s)
        nc._nrt_pseudo_barrier()

    # ring credits: pre-seed nb so the first nb ops DMAs don't wait
    nc.tensor.sem_inc(pe_sem, nb)

    class _Ctr:
        """Cumulative wait target: a per-engine register, or (debug,
        reps==1 only) a compile-time constant."""
        def __init__(self, eng, name, init):
            self.eng, self.val = eng, init
            if use_regs:
                self.reg = eng.alloc_register(name)
                eng.reg_mov(self.reg, init)
        def add(self, d):
            self.val += d
            if use_regs:
                self.eng.reg_add(self.reg, self.reg, d)
        def wait(self, sem):
            self.eng.wait_ge(sem, self.reg if use_regs else self.val)
        def bump_to(self, target):
            # target sequences are iteration-invariant in their deltas,
            # so the emitted reg_adds replay correctly every loop pass
            assert target >= self.val, (target, self.val)
            if target > self.val:
                self.add(target - self.val)

    if not use_regs:
        assert reps == 1
    r_credit = _Ctr(nc.sync, "r_credit", 0)
    if nb == ng or pair > 1:
        # one DMA per lane per iteration -> every lane's target is the
        # same 16*(iter+1): share a single counter, bumped once per iter.
        r_dma_iter = _Ctr(nc.tensor, "r_dma", 16)
        r_dma = None
    else:
        r_dma = [_Ctr(nc.tensor, f"r_dma{l}", 0) for l in range(nb)]
    r_xt = _Ctr(nc.tensor, "r_xt", 16)
    r_actpe = _Ctr(nc.tensor, "r_actpe", 0)
    r_xtfree = _Ctr(nc.scalar, "r_xtfree", nb)
    r_act = _Ctr(nc.scalar, "r_act", 0)
    r_pe = _Ctr(nc.vector, "r_pe", nb)
    r_out = _Ctr(nc.vector, "r_out", 0)

    act_chunks = []   # (qi, c0, c1) in DVE issue order, 2 per qi; flip
    for qi in range(QCH):   # columns only — nf tail columns go via out_nf
        for h in range(2):
            act_chunks.append((qi, c_flip * h // 2, c_flip * (h + 1) // 2))

    assert np.gcd(slot_stride, nb) == 1
    slot_of = lambda s: (s % nb) * slot_stride % nb

    with nc.Fori(0, reps, 1):
        # --- sync: ops stream into the ring ---
        if do_ops_dma and pair > 1:
            for d in range(ng_dma):
                # slot d's buffers hold groups [d*pair, min((d+1)*pair, ng))
                r_credit.bump_to(min(pair * (d + 1), ng))
                r_credit.wait(pe_sem)
                w = (min(pair * (d + 1), ng) - pair * d) * gsz
                nc.sync.dma_start(
                    mring[:, d * gszp:d * gszp + w], ops_dram[d][:, :w]
                ).then_inc(dma_sems[d], 16)
        elif do_ops_dma:
            for g in range(ng):
                r_credit.add(1)
                r_credit.wait(pe_sem)
                slot = slot_of(g)
                if ops_engine == "alt":
                    eng = nc.sync if g % 2 == 0 else nc.scalar
                else:
                    eng = getattr(nc, ops_engine)
                eng.dma_start(
                    mring[:, slot * gsz:(slot + 1) * gsz], ops_dram[g]
                ).then_inc(dma_sems[slot], 16)

        # --- scalar: xt in, outs out ---
        r_xtfree.wait(pe_sem)
        if xt_chunks == 1:
            nc.scalar.dma_start(xt_sb, xt_dram).then_inc(xt_sem, 16)
        else:
            w = QCH * ncol
            for c in range(xt_chunks):
                a, b = w * c // xt_chunks, w * (c + 1) // xt_chunks
                nc.scalar.dma_start(xt_sb[:, a:b], xt_dram[:, a:b]
                                    ).then_inc(xt_sem, 16)
        if do_out and do_act:
            for qi in range(QCH):
                r_act.add(2)
                r_act.wait(act_sem)
                nc.scalar.dma_start(
                    out_dram[:, qi * ncol:qi * ncol + c_flip],
                    o_sb[:, qi * ncol:qi * ncol + c_flip]
                ).then_inc(out_sem, 16)
            for k in range(nf):
                u = k % 4
                cw = maxc[ng - nf + k]
                r_act.bump_to(9 + k)   # act inc of nf relu k (iter-0 vals)
                r_act.wait(act_sem)
                nc.scalar.dma_start(
                    out_nf_dram[noffs[k]:noffs[k] + cw, :],
                    o_nf[0:cw, u * D:(u + 1) * D]
                ).then_inc(out_sem, 16)
        r_xtfree.add(ng)

        # --- tensor: the matmul stream ---
        r_xt.wait(xt_sem)
        r_actpe.wait(act_sem)
        for s in range(ng):
            if do_mm:
                if pair > 1:
                    d, t = divmod(s, pair)
                    if do_ops_dma and t == 0:
                        r_dma_iter.wait(dma_sems[d])
                    mbase = d * gszp + t * gsz
                else:
                    lane = slot_of(s)
                    if nb != ng:
                        r_dma[lane].add(16)
                    if do_ops_dma:
                        (r_dma_iter if nb == ng else r_dma[lane]).wait(
                            dma_sems[lane])
                    mbase = lane * gsz
                slot = None
                cw = maxc[s]
                last = None
                k = s - (ng - nf)
                if nf_probe and s >= ng - nf_probe:
                    # TIMING PROBE ONLY (wrong math): non-flip shape —
                    # 4 matmuls/matrix, x columns stationary, matrix moving.
                    for qj in range(QCH):
                        lhsT = xt_sb[:, qj * ncol + offs[s]:
                                     qj * ncol + offs[s] + cw]
                        rhs = mring[:, mbase + qj * (QCH * 128):
                                    mbase + (qj + 1) * (QCH * 128)]
                        last = nc.tensor.matmul(
                            ps_nf[0][0:cw, :], lhsT, rhs,
                            start=(qj == 0), stop=(qj == QCH - 1))
                elif k >= 0:
                    # non-flip: x columns stationary, matrix rows moving.
                    # PSUM: 4 dedicated banks, reused every 4 nf slots;
                    # a bank is freed by the eager DVE relu of slot k-4
                    # (act inc #(8+(k-4)+1) of this iteration).
                    b = k % 4
                    if k >= 4:
                        r_actpe.bump_to(k + 5)   # iter-0 target 8+(k-4)+1
                        r_actpe.wait(act_sem)
                    for qj in range(QCH):
                        lhsT = xt_sb[:, qj * ncol + offs[s]:
                                     qj * ncol + offs[s] + cw]
                        rhs = mring[:, mbase + qj * D:
                                    mbase + (qj + 1) * D]
                        last = nc.tensor.matmul(
                            ps_nf[b][0:cw, :], lhsT, rhs,
                            start=(qj == 0), stop=(qj == QCH - 1))
                else:
                    for qi in range(QCH):
                        for qj in range(QCH):
                            ck = mbase + (qj * QCH + qi) * 128
                            lhsT = mring[:, ck:ck + 128]
                            rhs = xt_sb[:, qj * ncol + offs[s]:
                                        qj * ncol + offs[s] + cw]
                            last = nc.tensor.matmul(
                                ps[qi][:, offs[s]:offs[s] + cw], lhsT, rhs,
                                start=(qj == 0), stop=(qj == QCH - 1))
                last.then_inc(pe_sem, 1)
            else:
                nc.tensor.sem_inc(pe_sem, 1)
        r_xt.add(16)
        r_actpe.bump_to(n_act)
        if nb == ng:
            r_dma_iter.add(16)

        # --- vector: relu PSUM -> SBUF ---
        if do_act:
            r_pe.bump_to(nb + ng - nf)   # all flip slots done
            r_pe.wait(pe_sem)
            r_out.wait(out_sem)          # prev iteration's outs landed
            for qi, c0, c1 in act_chunks:
                nc.vector.tensor_scalar_max(
                    o_sb[:, qi * ncol + c0:qi * ncol + c1],
                    ps[qi][:, c0:c1], 0.0
                ).then_inc(act_sem, 1)
            for k in range(nf):
                u = k % 4
                cw = maxc[ng - nf + k]
                r_pe.bump_to(nb + ng - nf + k + 1)   # nf slot k done
                r_pe.wait(pe_sem)
                if k >= 4:
                    # o_nf column position reused: out DMA of slot k-4
                    # (out #(4+(k-4)+1) of this iteration) must have landed
                    r_out.bump_to(16 * (k + 1))
                    r_out.wait(out_sem)
                nc.vector.tensor_scalar_max(
                    o_nf[0:cw, u * D:(u + 1) * D],
                    ps_nf[u][0:cw, :], 0.0
                ).then_inc(act_sem, 1)
            r_pe.bump_to(nb + ng)
            r_out.bump_to(16 * n_out)
        else:
            r_pe.add(ng)
            for _ in range(n_act):
                nc.vector.sem_inc(act_sem, 1)

    # quiesce: last iteration's out DMAs must have landed, then zero the
    # sems so a re-execution of this NEFF starts from clean state (waits
    # use absolute monotonic targets).
    if do_out and do_act:
        nc.scalar.wait_ge(out_sem, 64 * reps)
    nc.all_engine_barrier()
    for s in sems:
        nc.sync.sem_clear(s)
    nc.all_engine_barrier()
    nc.compile()
    return nc


def _route(attrs):
    """Group sample indices by attribute, chunk to <=128, snake-balance
    across cores. Returns per-core slot lists of (attr_id, idx_array),
    each list sorted by descending group size."""
    order = np.argsort(attrs, kind="stable")
    sorted_attrs = attrs[order]
    uniq, starts, counts = np.unique(sorted_attrs, return_index=True,
                                     return_counts=True)
    chunks = []
    for a, st, c in zip(uniq, starts, counts):
        idx = order[st:st + c]
        for o in range(0, c, 128):
            chunks.append((int(a), idx[o:o + 128]))
    chunks.sort(key=lambda t: -len(t[1]))
    per_core = [[] for _ in range(N_CORES)]
    for i, ch in enumerate(chunks):
        r, pos = divmod(i, N_CORES)
        k = pos if r % 2 == 0 else N_CORES - 1 - pos
        per_core[k].append(ch)
    return per_core


def _layout(per_core, align=1):
    """Per-slot-rank column capacity/offset shared by all cores.

    align: round capacities up so every slot's column offset is a multiple
    of `align` (align=2 makes f32 PSUM writes 8B-cacheline-aligned).
    """
    nm = max(1, max(len(s) for s in per_core))
    maxc = [1] * nm
    for slots in per_core:
        for s, (_, idx) in enumerate(slots):
            maxc[s] = max(maxc[s], len(idx))
    maxc = [-(-c // align) * align for c in maxc]
    offs = [0] * nm
    for s in range(1, nm):
        offs[s] = offs[s - 1] + maxc[s - 1]
    ncol = offs[-1] + maxc[-1]
    return nm, maxc, offs, ncol


def _prepare(attrs, objs, attr_ops, obj_emb, orient="flip", pair=None,
             align=None):
    """Route + build per-core device input maps."""
    if pair is None:
        pair = PAIR
    if align is None:
        align = ALIGN
    per_core = _route(attrs)
    nm, maxc, offs, ncol = _layout(per_core, align=align)
    nmp = -(-nm // pair) * pair

    rep = obj_emb[objs] * np.float32(1.0 / A_SCALE)  # [B, D], 1/s folded in
    ng = nmp // pair
    # raw hybrid: the last NF slots are computed non-flip on-device and
    # need the row-major (moving-operand) layout instead
    nf_eff = NF if (MODE == "raw" and orient == "flip") else 0
    in_maps = []
    for k in range(N_CORES):
        slots = per_core[k]
        ops_t = np.zeros((ng, 128, pair, QCH, QCH, 128), E3M4)
        r = np.zeros((ncol, D), np.float32)
        for s, (a, idx) in enumerate(slots):
            g, t = divmod(s, pair)
            at = np.clip(attr_ops[a].T * A_SCALE, -15.5, 15.5).astype(E3M4)
            if orient == "flip" and s < nm - nf_eff:
                # ops_t[g, p, t, qj, qi, i] = s*A[qi*128+i, qj*128+p]
                ops_t[g, :, t] = at.reshape(QCH, 128, QCH, 128).transpose(
                    1, 0, 2, 3)
            else:
                # ops_t[g, p, t, q, i] = s*A[i, q*128+p]
                ops_t[g, :, t] = at.reshape(QCH, 128, D).transpose(
                    1, 0, 2).reshape(128, QCH, QCH, 128)
            r[offs[s]:offs[s] + len(idx)] = rep[idx]
        # xt[p, q*ncol + c] = r[c, q*128 + p]
        xt = np.ascontiguousarray(r.reshape(ncol, QCH, 128).transpose(
            2, 1, 0).astype(np.float16)).reshape(128, -1)
        in_maps.append({"ops_t": ops_t.reshape(ng, 128, pair * QCH * D),
                        "xt": xt})
    return per_core, (nm, tuple(maxc), tuple(offs), ncol), in_maps


ORIENT = "flip"
MODE = "raw"      # "raw" (hand-managed sems) or "tile"


def _builder(reps=1, **kw):
    if MODE == "raw":
        def build(maxc, offs, ncol, **kw2):
            return _build_raw(list(maxc), list(offs), ncol, reps=reps,
                              **{**kw, **kw2})
    else:
        def build(maxc, offs, ncol, **kw2):
            b = _build_nc_flip if ORIENT == "flip" else _build_nc
            return b(list(maxc), list(offs), ncol, reps=reps, pair=PAIR,
                     **{**kw, **kw2})
    return build


def build_timing(maxc, offs, ncol, reps):
    """test.py hook: build the looped variant for wall-delta timing."""
    return _builder(reps=reps)(maxc, offs, ncol)


def kernel(attrs, objs, attr_ops, obj_emb):
    global LAST_RESULTS
    attrs = np.asarray(attrs)
    objs = np.asarray(objs)
    attr_ops = np.asarray(attr_ops, dtype=np.float32)
    obj_emb = np.asarray(obj_emb, dtype=np.float32)
    B = attrs.shape[0]
    d = obj_emb.shape[1]
    assert d == D and attr_ops.shape[1:] == (D, D)

    per_core, (nm, maxc, offs, ncol), in_maps = _prepare(
        attrs, objs, attr_ops, obj_emb, orient=ORIENT)

    nc = _NC_CACHE.get((MODE, ORIENT, maxc))
    if nc is None:
        nc = _NC_CACHE[(MODE, ORIENT, maxc)] = _builder()(maxc, offs, ncol)

    res = run_bass_kernel_spmd(nc, in_maps, core_ids=list(range(N_CORES)),
                               trace=TRACE, trace_cores=TRACE_CORES)
    LAST_RESULTS = res

    nf_eff = NF if (MODE == "raw" and ORIENT == "flip") else 0
    noffs = [0]
    for s in range(nm - nf_eff, nm):
        noffs.append(noffs[-1] + maxc[s])
    out = np.zeros((B, d), np.float32)
    for k in range(N_CORES):
        out_k = res.results[k]["out"].astype(np.float32)
        if ORIENT == "flip":
            out_k = out_k.reshape(128, QCH, ncol).transpose(2, 1, 0).reshape(
                ncol, D)
        for s, (a, idx) in enumerate(per_core[k]):
            if s >= nm - nf_eff:
                kk = s - (nm - nf_eff)
                out_nf = res.results[k]["out_nf"].astype(np.float32)
                out[idx] = out_nf[noffs[kk]:noffs[kk] + len(idx)]
            else:
                out[idx] = out_k[offs[s]:offs[s] + len(idx)]
    return out



# revision 57
# speedup vs baseline: 1.0093x; 1.0093x over previous
"""Trainium2 Bass kernel for nn_AttributeOperator (MoE-style routing).

Computes out[b] = relu(attr_ops[attrs[b]] @ obj_emb[objs[b]]) for b in [0, B).

Strategy (expert-parallel): the dominant cost is streaming the attr_ops table
(N_ATTRS x D x D fp32 = 512 MB). Samples are grouped by attribute on the host,
groups are load-balanced across the 8 cores (snake deal by group size), and
each core streams only its own subset of operator matrices from HBM exactly
once, quantized on the host to fp8 e3m4 (TRN FP8_EXP3, 4 mantissa bits) with
a global x128 scale folded into the fp16 x vectors — 1 byte/elem halves the
HBM stream vs fp16 (rel err 1.39e-2 vs the f32 reference, under the 2e-2
gate; e4m3 fails at 3.2e-2). 63 matrices/core = 16.5 MB -> 46.2 us at the
358 GB/s per-core HBM cap; +xt/out traffic the HBM floor is ~48.2 us.

MODE="raw" (current, ~51.5 us/iter): hand-scheduled bass, no Tile
scheduling. Measured motivation: Tile attaches a sem update to every matmul
(they serialize at ~26 ns at the EVT_SEM register), inserts a full
drain+reset barrier between loop iterations (~4 us), and the PE+DMA streams
ended up only ~40% overlapped (66.5 us total vs 46 us DMA-alone). The raw
version:
  * credit/token semaphores whose wait targets live in per-engine registers
    (monotonic, iteration-invariant deltas) -> NO inter-iteration barrier;
    the 63-slot ops ring spans loop iterations and the act/out tail of
    iteration i overlaps the DMA stream of iteration i+1;
  * one sem inc per GROUP (then_inc on its last matmul; sound: PE
    completions are pc-ordered), one per-slot DMA-completion sem LANE
    (a single shared sem is UNSOUND with >1 DMA in flight: the 16 per-DMA
    increments come from 16 independent SDMA engines and interleave);
  * hybrid orientation: first 46 slots flip (A^T chunks stationary, fp8
    fast-weight-load, 16 matmuls/matrix, accumulating out^T in 4 PSUM
    banks), last NF=17 slots non-flip (x columns stationary, matrix rows
    moving, 4 matmuls/matrix into 4 dedicated PSUM banks cycled with
    eager per-slot DVE relu). Measured: a concurrent SDMA stream is slowed
    ~9 us by the flip-only PE instruction stream (1008 instr); the hybrid
    (756+68 instr, PE busy still < DMA) cuts that to ~3 us. All-non-flip
    is PE-bound (54 us) and slower. NB tile_position col-packing for the
    nf PSUM crashes the exec unit (NRT 101) — use whole banks only.
Sems are zeroed at kernel end so NEFF re-executions start clean; sem lanes
use nb=63 (= ng) so every lane has exactly one DMA/iteration and all lanes
share one wait-target register.

Known dead ends (measured, do not retry blindly):
  * pair=2 DMA batching (4 KB/partition runs, ~0.5-1 us upside): the first
    EXECUTION of the NEFF corrupts the first two non-flip slots (one stale
    qj chunk) even though the emitted waits/addresses are correct;
    re-executions are silently "correct" only because stale SBUF equals the
    previous execution's identical weights. Genuine read-before-land race,
    root cause unidentified -> pair stays 1.
  * sem_clear at kernel START (gpsimd + _nrt_pseudo_barrier or
    + all_engine_barrier): races the engine streams, rel err ~0.3-0.8.
  * ALIGN=2 layout: rel err 0.68 (latent 4-alignment assumption somewhere
    in the raw path).
  * tile_position col-packing for nf PSUM: NRT 101 exec-unit crash.

Previous Tile-based implementation (MODE="tile", ~64.6-66.5 us) is kept
for A/B; its ablations: fp16 streaming 108 us, fp8 DMA alone 47 us, PE
alone 39 us. The SPMD program is identical on all 8 cores; only per-core
tensors differ. Slot s has a fixed column capacity maxc[s] = max over
cores of that rank's group size, so the one program fits every core's
routing. Timing method (test.py): paired hardware-loop wall-delta,
(wall(R=2001) - wall(R=201)) / 1800.
"""

import numpy as np
import ml_dtypes

import concourse.tile as tile
from concourse import bacc, mybir
from concourse.bass_utils import run_bass_kernel_spmd

N_CORES = 8
D = 512               # embedding dim (hardcoded per problem spec)
QCH = D // 128        # contraction chunks of 128 partitions
# attr_ops stream is fp8 e3m4 (TRN FP8_EXP3): normals cover [0.25, 15.5], so
# scale A up by 128 (|A|max ~0.11 -> ~13.9) and fold 1/128 into x on the host.
A_SCALE = 128.0
E3M4 = ml_dtypes.float8_e3m4

# test.py hooks (ignored by the grading harness)
LAST_RESULTS = None   # BassKernelResults of the most recent run
TRACE = False
TRACE_CORES = None

PAIR = 1
# Slot column offsets aligned to 4 -> every matmul's f32 PSUM write starts
# 16B-cacheline-aligned and every fp16 xt read 8B-aligned (PSUM lines are
# 8B, SBUF lines 16B; misaligned partial-line PSUM writes measurably slow
# the matmul stream: align=4 beat align=1 by ~5us/iter in-process).
ALIGN = 4
_NC_CACHE = {}


def _build_nc(maxc, offs, ncol, ops_bufs=8, pair=1, sync_frac=(1, 1), reps=1,
              out_engine="scalar", staggered=False, relu_engine="scalar",
              xt_engine="scalar", ops_dt="f8e3", out_dt="f16",
              do_ops_dma=True, do_mm=True, do_act=True, do_out=True):
    """Build + compile the SPMD program.

    maxc[s]: column capacity of slot s; offs[s]: column offset of slot s;
    ncol: total columns (= offs[-1] + maxc[-1]).
    pair: matrices loaded per ops DMA (amortizes per-DMA fixed costs).
    sync_frac: (a, b) -> a of every b ops DMAs issue on sync, rest on scalar.
    reps: hardware-loop repetitions of the whole kernel (for timing).
    staggered: staggered-reset loop back-edge — wedges this device, keep False.
    """
    nm = len(maxc)
    nmp = -(-nm // pair) * pair  # nm rounded up to a multiple of pair
    ng = nmp // pair
    mdt = {"f8e3": mybir.dt.float8e3, "f8e4": mybir.dt.float8e4,
           "f16": mybir.dt.float16}[ops_dt]
    odt = {"f16": mybir.dt.float16, "f32": mybir.dt.float32}[out_dt]
    nc = bacc.Bacc("TRN2", target_bir_lowering=False, debug=False,
                   num_devices=N_CORES)
    # per-group layout [p, t, q, i]: each partition's data is one contiguous
    # pair*QCH*D-element run -> one big DMA descriptor per partition
    ops_dram = nc.dram_tensor("ops_t", [ng, 128, pair * QCH * D],
                              mdt, kind="ExternalInput").ap()
    xt_dram = nc.dram_tensor("xt", [128, QCH * ncol], mybir.dt.float16,
                             kind="ExternalInput").ap()
    out_dram = nc.dram_tensor("out", [ncol, D], odt,
                              kind="ExternalOutput").ap()

    with tile.TileContext(nc) as tc:
        with (
            tc.tile_pool(name="xt", bufs=1) as xt_pool,
            tc.tile_pool(name="ops", bufs=ops_bufs) as ops_pool,
            tc.tile_pool(name="ps", bufs=8, space="PSUM") as ps_pool,
            tc.tile_pool(name="o", bufs=4) as o_pool,
        ):
            def body():
                xt_sb = xt_pool.tile([128, QCH * ncol], mybir.dt.float16)
                getattr(nc, xt_engine).dma_start(xt_sb[:], xt_dram[:])
                if not do_ops_dma:
                    m0 = ops_pool.tile([128, pair * QCH * D], mdt, tag="m")
                    nc.sync.dma_start(m0[:], ops_dram[0])

                for g in range(ng):
                    if do_ops_dma:
                        m = ops_pool.tile([128, pair * QCH * D], mdt, tag="m")
                        issuer = nc.sync if g % sync_frac[1] < sync_frac[0] \
                            else nc.scalar
                        issuer.dma_start(m[:], ops_dram[g])
                    else:
                        m = m0
                    for t in range(pair):
                        s = g * pair + t
                        if s >= nm:
                            break
                        cw = maxc[s]
                        if not do_mm:
                            continue
                        ps = ps_pool.tile([cw, D], mybir.dt.float32, tag="ps")
                        for q in range(QCH):
                            lhsT = xt_sb[:, q * ncol + offs[s]:
                                         q * ncol + offs[s] + cw]
                            rhs = m[:, (t * QCH + q) * D:
                                    (t * QCH + q + 1) * D]
                            nc.tensor.matmul(ps[:], lhsT, rhs,
                                             start=(q == 0),
                                             stop=(q == QCH - 1))
                        if not do_act:
                            continue
                        o = o_pool.tile([cw, D], odt, tag="o")
                        if relu_engine == "vector":
                            nc.vector.tensor_scalar_max(o[:], ps[:], 0.0)
                        else:
                            nc.scalar.activation(
                                o[:], ps[:], mybir.ActivationFunctionType.Relu)
                        if not do_out:
                            continue
                        out_eng = getattr(nc, out_engine)
                        out_eng.dma_start(
                            out_dram[offs[s]:offs[s] + cw, :], o[:])

            if reps == 1:
                body()
            else:
                with tc.For_i(0, reps, 1,
                              hint_engines=(mybir.EngineType.PE,),
                              staggered_reset=staggered):
                    body()

    nc.compile()
    return nc


def _build_nc_flip(maxc, offs, ncol, ops_bufs=8, pair=1, sync_frac=(1, 1),
                   reps=1, out_engine="scalar", staggered=False,
                   relu_engine="vector", xt_engine="scalar", ops_dt="f8e3",
                   out_dt="f16", do_ops_dma=True, do_mm=True, do_act=True,
                   do_out=True, mm_src="real", mm_every=1, dma_split=False,
                   ops_engine=None, act_split=2, xt_split=True,
                   out_per_qi=True, mm_order="qi", mm_split=1,
                   ops_frac=1, prog_out=0):
    """Flipped orientation: A chunks are the stationary operand (fp8 weights
    -> fast weight load), x columns stream as the moving operand.

    Per slot s (one operator matrix A), for each output chunk qi and
    contraction chunk qj: ldweights(A^T[qj,qi] 128x128) + matmul over the
    slot's cw x-columns, accumulating out^T[qi*128:+128, cols(s)] in a PSUM
    tile [128, ncol] shared by all slots. One ReLU per qi over the full
    [128, ncol] bank, one contiguous output DMA of out^T.
    """
    nm = len(maxc)
    nmp = -(-nm // pair) * pair
    ng = nmp // pair
    mdt = {"f8e3": mybir.dt.float8e3, "f8e4": mybir.dt.float8e4,
           "f16": mybir.dt.float16}[ops_dt]
    odt = {"f16": mybir.dt.float16, "f32": mybir.dt.float32}[out_dt]
    nc = bacc.Bacc("TRN2", target_bir_lowering=False, debug=False,
                   num_devices=N_CORES)
    # ops_t[g, p, ((t*QCH+qj)*QCH+qi)*128 + i] = s*A_s[qi*128+i, qj*128+p]
    # ops_frac>1: timing-probe mode — stream only 1/ops_frac of the bytes
    # (results are garbage; used to measure DMA-vs-PE scaling).
    gsz = pair * QCH * QCH * 128 // ops_frac
    ops_dram = nc.dram_tensor("ops_t", [ng, 128, gsz],
                              mdt, kind="ExternalInput").ap()
    xt_dram = nc.dram_tensor("xt", [128, QCH * ncol], mybir.dt.float16,
                             kind="ExternalInput").ap()
    # out^T: out_dram[p, qi*ncol + c] = out[c, qi*128+p]
    out_dram = nc.dram_tensor("out", [128, QCH * ncol], odt,
                              kind="ExternalOutput").ap()

    with tile.TileContext(nc) as tc:
        with (
            tc.tile_pool(name="xt", bufs=1) as xt_pool,
            tc.tile_pool(name="ops", bufs=ops_bufs) as ops_pool,
            tc.tile_pool(name="m0p", bufs=1) as m0_pool,
            tc.tile_pool(name="ps", bufs=8, space="PSUM") as ps_pool,
            tc.tile_pool(name="o", bufs=2) as o_pool,
        ):
            def body():
                xt_sb = xt_pool.tile([128, QCH * ncol], mybir.dt.float16)
                if xt_split:
                    for qj in range(QCH):
                        getattr(nc, xt_engine).dma_start(
                            xt_sb[:, qj * ncol:(qj + 1) * ncol],
                            xt_dram[:, qj * ncol:(qj + 1) * ncol])
                else:
                    getattr(nc, xt_engine).dma_start(xt_sb[:], xt_dram[:])
                ps = [ps_pool.tile([128, ncol], mybir.dt.float32, tag="ps",
                                   name=f"ps{qi}")
                      for qi in range(QCH)]
                if not do_ops_dma or mm_src == "m0":
                    m0 = m0_pool.tile([128, gsz], mdt, tag="m0", bufs=1)
                    nc.sync.dma_start(m0[:], ops_dram[0])

                # progressive act/out: emit ReLU + out DMA for a column span
                # once the last slot covering it has been multiplied, instead
                # of serially after the whole stream.
                span_end = {}           # slot s -> (c0, c1) to flush after s
                o_prog = None
                if prog_out and do_act:
                    o_prog = o_pool.tile([128, QCH * ncol], odt, tag="o",
                                         bufs=1)
                    bounds = [nm * (j + 1) // prog_out - 1
                              for j in range(prog_out)]
                    c_prev = 0
                    for s_e in bounds:
                        c_hi = offs[s_e] + maxc[s_e]
                        span_end[s_e] = (c_prev, c_hi)
                        c_prev = c_hi

                def flush_span(c0, c1):
                    for qi in range(QCH):
                        dst = o_prog[:, qi * ncol + c0:qi * ncol + c1]
                        src = ps[qi][:, c0:c1]
                        if relu_engine == "vector":
                            nc.vector.tensor_scalar_max(dst, src, 0.0)
                        else:
                            nc.scalar.activation(
                                dst, src, mybir.ActivationFunctionType.Relu)
                        if do_out:
                            getattr(nc, out_engine).dma_start(
                                out_dram[:, qi * ncol + c0:qi * ncol + c1],
                                dst)

                for g in range(ng):
                    if do_ops_dma:
                        m = ops_pool.tile([128, gsz], mdt, tag="m")
                        if dma_split:
                            h = gsz // 2
                            nc.sync.dma_start(m[:, :h], ops_dram[g][:, :h])
                            nc.scalar.dma_start(m[:, h:], ops_dram[g][:, h:])
                        elif ops_engine is not None:
                            getattr(nc, ops_engine).dma_start(
                                m[:], ops_dram[g])
                        else:
                            issuer = nc.sync \
                                if g % sync_frac[1] < sync_frac[0] \
                                else nc.scalar
                            issuer.dma_start(m[:], ops_dram[g])
                        if mm_src == "m0":
                            m = m0
                    else:
                        m = m0
                    for t in range(pair):
                        s = g * pair + t
                        if s >= nm:
                            break
                        cw = maxc[s]
                        if not do_mm or s % mm_every:
                            continue
                        order = [(qi, qj) for qi in range(QCH)
                                 for qj in range(QCH)] \
                            if mm_order == "qi" else \
                            [(qi, qj) for qj in range(QCH)
                             for qi in range(QCH)]
                        for qi, qj in order:
                            ck = (((t * QCH + qj) * QCH + qi)
                                  % (gsz // 128)) * 128
                            lhsT = m[:, ck:ck + 128]
                            for h in range(mm_split):
                                a0 = offs[s] + cw * h // mm_split
                                a1 = offs[s] + cw * (h + 1) // mm_split
                                if a1 == a0:
                                    continue
                                rhs = xt_sb[:, qj * ncol + a0:
                                            qj * ncol + a1]
                                nc.tensor.matmul(
                                    ps[qi][:, a0:a1],
                                    lhsT, rhs, start=(qj == 0),
                                    stop=(qj == QCH - 1))
                if not do_act:
                    return
                o = o_pool.tile([128, QCH * ncol], odt, tag="o")
                for qi in range(QCH):
                    for h in range(act_split):
                        c0 = ncol * h // act_split
                        c1 = ncol * (h + 1) // act_split
                        dst = o[:, qi * ncol + c0:qi * ncol + c1]
                        src = ps[qi][:, c0:c1]
                        if relu_engine == "vector":
                            nc.vector.tensor_scalar_max(dst, src, 0.0)
                        else:
                            nc.scalar.activation(
                                dst, src, mybir.ActivationFunctionType.Relu)
                    if do_out and out_per_qi:
                        getattr(nc, out_engine).dma_start(
                            out_dram[:, qi * ncol:(qi + 1) * ncol],
                            o[:, qi * ncol:(qi + 1) * ncol])
                if do_out and not out_per_qi:
                    getattr(nc, out_engine).dma_start(out_dram[:], o[:])

            if reps == 1:
                body()
            else:
                with tc.For_i(0, reps, 1,
                              hint_engines=(mybir.EngineType.PE,),
                              staggered_reset=staggered):
                    body()

    nc.compile()
    return nc


NF = 17   # tail slots computed in non-flip orientation (hybrid)


def _build_raw(maxc, offs, ncol, reps=1, nb=63, do_ops_dma=True, do_mm=True,
               do_act=True, do_out=True, xt_chunks=1, use_regs=True,
               do_clear=False, ops_engine="sync", slot_stride=1,
               nf_probe=0, nf=None, pair=1):
    """Raw-bass (no Tile scheduling) flip-orientation kernel.

    Motivation (measured): Tile attaches a semaphore update to every matmul
    (1008/iter) and a full engine drain+reset between loop iterations; the
    per-instruction sem updates serialize at ~26ns each at the EVT_SEM
    register and the PE+DMA streams end up only ~40% overlapped (base 66.5us
    vs 46us DMA-alone line rate). This version hand-manages semaphores:
    one inc per ops-DMA (hardware, +16), one inc per GROUP on PE (last
    matmul's then_inc, sound because PE completions are pc-ordered), and a
    credit scheme whose wait targets are carried in per-engine registers so
    the hardware loop needs NO inter-iteration barrier: the ops-DMA ring
    spans iteration boundaries and the act/out tail of iteration i overlaps
    the DMA stream of iteration i+1.

    Engine program (per iteration):
      sync  : ng x [credit wait on pe_sem; dma_start ops -> ring (+16 dma_sem)]
      tensor: wait xt_sem; wait act_sem (PSUM free);
              ng x [wait dma_sem; 16 matmuls; last +1 pe_sem]
      scalar: wait pe_sem (xt free); dma xt (+16 xt_sem);
              4 x [wait act_sem; dma out qi (+16 out_sem)]
      vector: wait pe_sem (all groups); wait out_sem (o free);
              8 x relu chunk (+1 act_sem each)
    """
    nm = len(maxc)
    ng = nm
    if nf is None:
        nf = 0 if nf_probe else NF
    nf = min(nf, ng)
    assert all(maxc[s] <= 32 for s in range(ng - nf, ng))
    c_flip = offs[ng - nf] if nf else ncol      # first non-flip column
    noffs = [0]                                  # out_nf row offsets
    for s in range(ng - nf, ng):
        noffs.append(noffs[-1] + maxc[s])
    n_out = 4 + nf                               # out DMAs per iteration
    n_act = 8 + nf                               # act_sem incs per iteration

    nc = bacc.Bacc("TRN2", target_bir_lowering=False, debug=False,
                   num_devices=N_CORES)
    gsz = QCH * QCH * 128
    # pair>1: `pair` matrices per DMA (bigger per-partition runs amortize
    # SDMA packet overhead); the last DMA is truncated so padding matrices
    # are never transferred. Forces whole-iteration ring (nb = ng).
    ng_dma = -(-ng // pair)
    gszp = pair * gsz
    if pair > 1:
        nb = ng
    ops_dram = nc.dram_tensor("ops_t", [ng_dma, 128, gszp],
                              mybir.dt.float8e3,
                              kind="ExternalInput").ap()
    xt_dram = nc.dram_tensor("xt", [128, QCH * ncol], mybir.dt.float16,
                             kind="ExternalInput").ap()
    out_dram = nc.dram_tensor("out", [128, QCH * ncol], mybir.dt.float16,
                              kind="ExternalOutput").ap()
    out_nf_dram = nc.dram_tensor(
        "out_nf", [max(1, noffs[-1]), D], mybir.dt.float16,
        kind="ExternalOutput").ap()

    xt_sb = nc.alloc_sbuf_tensor("xt_sb", [128, QCH * ncol],
                                 mybir.dt.float16).ap()
    o_sb = nc.alloc_sbuf_tensor("o_sb", [128, QCH * ncol],
                                mybir.dt.float16).ap()
    o_nf = nc.alloc_sbuf_tensor("o_nf", [128, 4 * D], mybir.dt.float16).ap()
    mring = nc.alloc_sbuf_tensor("mring", [128, ng_dma * gszp],
                                 mybir.dt.float8e3).ap()
    ps = [nc.alloc_psum_tensor(f"ps{qi}", [128, ncol], mybir.dt.float32).ap()
          for qi in range(QCH)]
    # 4 PSUM banks for the non-flip tail, cycled with reuse distance 4
    # (no tile_position partition packing: that path crashes the exec unit)
    ps_nf = [nc.alloc_psum_tensor(f"ps_nf{b}", [128, D],
                                  mybir.dt.float32).ap()
             for b in range(4)] if (nf or nf_probe) else None

    # dma completion sems: one LANE per ring slot. A single shared sem is
    # unsound with >1 DMA in flight: the 16 per-DMA increments come from 16
    # independent SDMA engines, so increments of DMA k+1 can stand in for
    # laggards of DMA k and a threshold wait passes before k fully lands
    # (observed: corruption starting exactly where PE catches the stream).
    # With lane=slot and ring credits bounding in-flight <= nb, each lane
    # has at most one DMA outstanding -> its count is unambiguous.
    # nb must divide ng so lane assignment is iteration-invariant.
    assert ng % nb == 0, (ng, nb)
    n_lanes = ng_dma if pair > 1 else nb
    dma_sems = [nc.alloc_semaphore(f"dma_sem{l}") for l in range(n_lanes)]
    xt_sem = nc.alloc_semaphore("xt_sem")
    pe_sem = nc.alloc_semaphore("pe_sem")
    act_sem = nc.alloc_semaphore("act_sem")
    out_sem = nc.alloc_semaphore("out_sem")
    sems = dma_sems + [xt_sem, pe_sem, act_sem, out_sem]

    # NB: do NOT clear sems at kernel start — both a gpsimd sem_clear +
    # _nrt_pseudo_barrier and a sem_clear + all_engine_barrier preamble
    # measurably RACE the engine streams here (verified: rel err jumps to
    # ~0.3-0.8). Instead sems are zeroed at kernel END (below), and the
    # program assumes zeroed sems at entry.
    if do_clear:
        for s in sems:
            nc.gpsimd.sem_clear(s)
        nc._nrt_pseudo_barrier()

    # ring credits: pre-seed nb so the first nb ops DMAs don't wait
    nc.tensor.sem_inc(pe_sem, nb)

    class _Ctr:
        """Cumulative wait target: a per-engine register, or (debug,
        reps==1 only) a compile-time constant."""
        def __init__(self, eng, name, init):
            self.eng, self.val = eng, init
            if use_regs:
                self.reg = eng.alloc_register(name)
                eng.reg_mov(self.reg, init)
        def add(self, d):
            self.val += d
            if use_regs:
                self.eng.reg_add(self.reg, self.reg, d)
        def wait(self, sem):
            self.eng.wait_ge(sem, self.reg if use_regs else self.val)
        def bump_to(self, target):
            # target sequences are iteration-invariant in their deltas,
            # so the emitted reg_adds replay correctly every loop pass
            assert target >= self.val, (target, self.val)
            if target > self.val:
                self.add(target - self.val)

    if not use_regs:
        assert reps == 1
    r_credit = _Ctr(nc.sync, "r_credit", 0)
    if nb == ng or pair > 1:
        # one DMA per lane per iteration -> every lane's target is the
        # same 16*(iter+1): share a single counter, bumped once per iter.
        r_dma_iter = _Ctr(nc.tensor, "r_dma", 16)
        r_dma = None
    else:
        r_dma = [_Ctr(nc.tensor, f"r_dma{l}", 0) for l in range(nb)]
    r_xt = _Ctr(nc.tensor, "r_xt", 16)
    r_actpe = _Ctr(nc.tensor, "r_actpe", 0)
    r_xtfree = _Ctr(nc.scalar, "r_xtfree", nb)
    r_act = _Ctr(nc.scalar, "r_act", 0)
    r_pe = _Ctr(nc.vector, "r_pe", nb)
    r_out = _Ctr(nc.vector, "r_out", 0)

    act_chunks = []   # (qi, c0, c1) in DVE issue order, 2 per qi; flip
    for qi in range(QCH):   # columns only — nf tail columns go via out_nf
        for h in range(2):
            act_chunks.append((qi, c_flip * h // 2, c_flip * (h + 1) // 2))

    assert np.gcd(slot_stride, nb) == 1
    slot_of = lambda s: (s % nb) * slot_stride % nb

    with nc.Fori(0, reps, 1):
        # --- sync: ops stream into the ring ---
        if do_ops_dma and pair > 1:
            for d in range(ng_dma):
                # slot d's buffers hold groups [d*pair, min((d+1)*pair, ng))
                r_credit.bump_to(min(pair * (d + 1), ng))
                r_credit.wait(pe_sem)
                w = (min(pair * (d + 1), ng) - pair * d) * gsz
                nc.sync.dma_start(
                    mring[:, d * gszp:d * gszp + w], ops_dram[d][:, :w]
                ).then_inc(dma_sems[d], 16)
        elif do_ops_dma:
            for g in range(ng):
                r_credit.add(1)
                r_credit.wait(pe_sem)
                slot = slot_of(g)
                if ops_engine == "alt":
                    eng = nc.sync if g % 2 == 0 else nc.scalar
                else:
                    eng = getattr(nc, ops_engine)
                eng.dma_start(
                    mring[:, slot * gsz:(slot + 1) * gsz], ops_dram[g]
                ).then_inc(dma_sems[slot], 16)

        # --- scalar: xt in, outs out ---
        r_xtfree.wait(pe_sem)
        if xt_chunks == 1:
            nc.scalar.dma_start(xt_sb, xt_dram).then_inc(xt_sem, 16)
        else:
            w = QCH * ncol
            for c in range(xt_chunks):
                a, b = w * c // xt_chunks, w * (c + 1) // xt_chunks
                nc.scalar.dma_start(xt_sb[:, a:b], xt_dram[:, a:b]
                                    ).then_inc(xt_sem, 16)
        if do_out and do_act:
            for qi in range(QCH):
                r_act.add(2)
                r_act.wait(act_sem)
                nc.scalar.dma_start(
                    out_dram[:, qi * ncol:qi * ncol + c_flip],
                    o_sb[:, qi * ncol:qi * ncol + c_flip]
                ).then_inc(out_sem, 16)
            for k in range(nf):
                u = k % 4
                cw = maxc[ng - nf + k]
                r_act.bump_to(9 + k)   # act inc of nf relu k (iter-0 vals)
                r_act.wait(act_sem)
                nc.scalar.dma_start(
                    out_nf_dram[noffs[k]:noffs[k] + cw, :],
                    o_nf[0:cw, u * D:(u + 1) * D]
                ).then_inc(out_sem, 16)
        r_xtfree.add(ng)

        # --- tensor: the matmul stream ---
        r_xt.wait(xt_sem)
        r_actpe.wait(act_sem)
        for s in range(ng):
            if do_mm:
                if pair > 1:
                    d, t = divmod(s, pair)
                    if do_ops_dma and t == 0:
                        r_dma_iter.wait(dma_sems[d])
                    mbase = d * gszp + t * gsz
                else:
                    lane = slot_of(s)
                    if nb != ng:
                        r_dma[lane].add(16)
                    if do_ops_dma:
                        (r_dma_iter if nb == ng else r_dma[lane]).wait(
                            dma_sems[lane])
                    mbase = lane * gsz
                slot = None
                cw = maxc[s]
                last = None
                k = s - (ng - nf)
                if nf_probe < 0 and s % 4 == 1:
                    # TIMING PROBE ONLY (wrong math): ~16 non-flip-shaped
                    # slots INTERLEAVED through the flip stream
                    for qj in range(QCH):
                        lhsT = xt_sb[:, qj * ncol + offs[s]:
                                     qj * ncol + offs[s] + cw]
                        rhs = mring[:, mbase + qj * (QCH * 128):
                                    mbase + (qj + 1) * (QCH * 128)]
                        last = nc.tensor.matmul(
                            ps_nf[(s // 4) % 4][0:cw, :], lhsT, rhs,
                            start=(qj == 0), stop=(qj == QCH - 1))
                elif nf_probe > 0 and s >= ng - nf_probe:
                    # TIMING PROBE ONLY (wrong math): non-flip shape —
                    # 4 matmuls/matrix, x columns stationary, matrix moving.
                    for qj in range(QCH):
                        lhsT = xt_sb[:, qj * ncol + offs[s]:
                                     qj * ncol + offs[s] + cw]
                        rhs = mring[:, mbase + qj * (QCH * 128):
                                    mbase + (qj + 1) * (QCH * 128)]
                        last = nc.tensor.matmul(
                            ps_nf[0][0:cw, :], lhsT, rhs,
                            start=(qj == 0), stop=(qj == QCH - 1))
                elif k >= 0:
                    # non-flip: x columns stationary, matrix rows moving.
                    # PSUM: 4 dedicated banks, reused every 4 nf slots;
                    # a bank is freed by the eager DVE relu of slot k-4
                    # (act inc #(8+(k-4)+1) of this iteration).
                    b = k % 4
                    if k >= 4:
                        r_actpe.bump_to(k + 5)   # iter-0 target 8+(k-4)+1
                        r_actpe.wait(act_sem)
                    for qj in range(QCH):
                        lhsT = xt_sb[:, qj * ncol + offs[s]:
                                     qj * ncol + offs[s] + cw]
                        rhs = mring[:, mbase + qj * D:
                                    mbase + (qj + 1) * D]
                        last = nc.tensor.matmul(
                            ps_nf[b][0:cw, :], lhsT, rhs,
                            start=(qj == 0), stop=(qj == QCH - 1))
                else:
                    for qi in range(QCH):
                        for qj in range(QCH):
                            ck = mbase + (qj * QCH + qi) * 128
                            lhsT = mring[:, ck:ck + 128]
                            rhs = xt_sb[:, qj * ncol + offs[s]:
                                        qj * ncol + offs[s] + cw]
                            last = nc.tensor.matmul(
                                ps[qi][:, offs[s]:offs[s] + cw], lhsT, rhs,
                                start=(qj == 0), stop=(qj == QCH - 1))
                last.then_inc(pe_sem, 1)
            else:
                nc.tensor.sem_inc(pe_sem, 1)
        r_xt.add(16)
        r_actpe.bump_to(n_act)
        if nb == ng:
            r_dma_iter.add(16)

        # --- vector: relu PSUM -> SBUF ---
        if do_act:
            r_pe.bump_to(nb + ng - nf)   # all flip slots done
            r_pe.wait(pe_sem)
            r_out.wait(out_sem)          # prev iteration's outs landed
            for qi, c0, c1 in act_chunks:
                nc.vector.tensor_scalar_max(
                    o_sb[:, qi * ncol + c0:qi * ncol + c1],
                    ps[qi][:, c0:c1], 0.0
                ).then_inc(act_sem, 1)
            for k in range(nf):
                u = k % 4
                cw = maxc[ng - nf + k]
                r_pe.bump_to(nb + ng - nf + k + 1)   # nf slot k done
                r_pe.wait(pe_sem)
                if k >= 4:
                    # o_nf column position reused: out DMA of slot k-4
                    # (out #(4+(k-4)+1) of this iteration) must have landed
                    r_out.bump_to(16 * (k + 1))
                    r_out.wait(out_sem)
                nc.vector.tensor_scalar_max(
                    o_nf[0:cw, u * D:(u + 1) * D],
                    ps_nf[u][0:cw, :], 0.0
                ).then_inc(act_sem, 1)
            r_pe.bump_to(nb + ng)
            r_out.bump_to(16 * n_out)
        else:
            r_pe.add(ng)
            for _ in range(n_act):
                nc.vector.sem_inc(act_sem, 1)

    # quiesce: last iteration's out DMAs must have landed, then zero the
    # sems so a re-execution of this NEFF starts from clean state (waits
    # use absolute monotonic targets).
    if do_out and do_act:
        nc.scalar.wait_ge(out_sem, 64 * reps)
    nc.all_engine_barrier()
    for s in sems:
        nc.sync.sem_clear(s)
    nc.all_engine_barrier()
    nc.compile()
    return nc


def _route(attrs):
    """Group sample indices by attribute, chunk to <=128, snake-balance
    across cores. Returns per-core slot lists of (attr_id, idx_array),
    each list sorted by descending group size."""
    order = np.argsort(attrs, kind="stable")
    sorted_attrs = attrs[order]
    uniq, starts, counts = np.unique(sorted_attrs, return_index=True,
                                     return_counts=True)
    chunks = []
    for a, st, c in zip(uniq, starts, counts):
        idx = order[st:st + c]
        for o in range(0, c, 128):
            chunks.append((int(a), idx[o:o + 128]))
    chunks.sort(key=lambda t: -len(t[1]))
    per_core = [[] for _ in range(N_CORES)]
    for i, ch in enumerate(chunks):
        r, pos = divmod(i, N_CORES)
        k = pos if r % 2 == 0 else N_CORES - 1 - pos
        per_core[k].append(ch)
    return per_core


def _layout(per_core, align=1):
    """Per-slot-rank column capacity/offset shared by all cores.

    align: round capacities up so every slot's column offset is a multiple
    of `align` (align=2 makes f32 PSUM writes 8B-cacheline-aligned).
    """
    nm = max(1, max(len(s) for s in per_core))
    maxc = [1] * nm
    for slots in per_core:
        for s, (_, idx) in enumerate(slots):
            maxc[s] = max(maxc[s], len(idx))
    maxc = [-(-c // align) * align for c in maxc]
    offs = [0] * nm
    for s in range(1, nm):
        offs[s] = offs[s - 1] + maxc[s - 1]
    ncol = offs[-1] + maxc[-1]
    return nm, maxc, offs, ncol


def _prepare(attrs, objs, attr_ops, obj_emb, orient="flip", pair=None,
             align=None):
    """Route + build per-core device input maps."""
    if pair is None:
        pair = PAIR
    if align is None:
        align = ALIGN
    per_core = _route(attrs)
    nm, maxc, offs, ncol = _layout(per_core, align=align)
    nmp = -(-nm // pair) * pair

    rep = obj_emb[objs] * np.float32(1.0 / A_SCALE)  # [B, D], 1/s folded in
    ng = nmp // pair
    # raw hybrid: the last NF slots are computed non-flip on-device and
    # need the row-major (moving-operand) layout instead
    nf_eff = NF if (MODE == "raw" and orient == "flip") else 0
    in_maps = []
    for k in range(N_CORES):
        slots = per_core[k]
        ops_t = np.zeros((ng, 128, pair, QCH, QCH, 128), E3M4)
        r = np.zeros((ncol, D), np.float32)
        for s, (a, idx) in enumerate(slots):
            g, t = divmod(s, pair)
            at = np.clip(attr_ops[a].T * A_SCALE, -15.5, 15.5).astype(E3M4)
            if orient == "flip" and s < nm - nf_eff:
                # ops_t[g, p, t, qj, qi, i] = s*A[qi*128+i, qj*128+p]
                ops_t[g, :, t] = at.reshape(QCH, 128, QCH, 128).transpose(
                    1, 0, 2, 3)
            else:
                # ops_t[g, p, t, q, i] = s*A[i, q*128+p]
                ops_t[g, :, t] = at.reshape(QCH, 128, D).transpose(
                    1, 0, 2).reshape(128, QCH, QCH, 128)
            r[offs[s]:offs[s] + len(idx)] = rep[idx]
        # xt[p, q*ncol + c] = r[c, q*128 + p]
        xt = np.ascontiguousarray(r.reshape(ncol, QCH, 128).transpose(
            2, 1, 0).astype(np.float16)).reshape(128, -1)
        in_maps.append({"ops_t": ops_t.reshape(ng, 128, pair * QCH * D),
                        "xt": xt})
    return per_core, (nm, tuple(maxc), tuple(offs), ncol), in_maps


ORIENT = "flip"
MODE = "raw"      # "raw" (hand-managed sems) or "tile"


def _builder(reps=1, **kw):
    if MODE == "raw":
        def build(maxc, offs, ncol, **kw2):
            return _build_raw(list(maxc), list(offs), ncol, reps=reps,
                              **{**kw, **kw2})
    else:
        def build(maxc, offs, ncol, **kw2):
            b = _build_nc_flip if ORIENT == "flip" else _build_nc
            return b(list(maxc), list(offs), ncol, reps=reps, pair=PAIR,
                     **{**kw, **kw2})
    return build


def build_timing(maxc, offs, ncol, reps):
    """test.py hook: build the looped variant for wall-delta timing."""
    return _builder(reps=reps)(maxc, offs, ncol)


def kernel(attrs, objs, attr_ops, obj_emb):
    global LAST_RESULTS
    attrs = np.asarray(attrs)
    objs = np.asarray(objs)
    attr_ops = np.asarray(attr_ops, dtype=np.float32)
    obj_emb = np.asarray(obj_emb, dtype=np.float32)
    B = attrs.shape[0]
    d = obj_emb.shape[1]
    assert d == D and attr_ops.shape[1:] == (D, D)

    per_core, (nm, maxc, offs, ncol), in_maps = _prepare(
        attrs, objs, attr_ops, obj_emb, orient=ORIENT)

    nc = _NC_CACHE.get((MODE, ORIENT, maxc))
    if nc is None:
        nc = _NC_CACHE[(MODE, ORIENT, maxc)] = _builder()(maxc, offs, ncol)

    res = run_bass_kernel_spmd(nc, in_maps, core_ids=list(range(N_CORES)),
                               trace=TRACE, trace_cores=TRACE_CORES)
    LAST_RESULTS = res

    nf_eff = NF if (MODE == "raw" and ORIENT == "flip") else 0
    noffs = [0]
    for s in range(nm - nf_eff, nm):
        noffs.append(noffs[-1] + maxc[s])
    out = np.zeros((B, d), np.float32)
    for k in range(N_CORES):
        out_k = res.results[k]["out"].astype(np.float32)
        if ORIENT == "flip":
            out_k = out_k.reshape(128, QCH, ncol).transpose(2, 1, 0).reshape(
                ncol, D)
        for s, (a, idx) in enumerate(per_core[k]):
            if s >= nm - nf_eff:
                kk = s - (nm - nf_eff)
                out_nf = res.results[k]["out_nf"].astype(np.float32)
                out[idx] = out_nf[noffs[kk]:noffs[kk] + len(idx)]
            else:
                out[idx] = out_k[offs[s]:offs[s] + len(idx)]
    return out



# revision 61
# speedup vs baseline: 1.0168x; 1.0075x over previous
"""Trainium2 Bass kernel for nn_AttributeOperator (MoE-style routing).

Computes out[b] = relu(attr_ops[attrs[b]] @ obj_emb[objs[b]]) for b in [0, B).

Strategy (expert-parallel): the dominant cost is streaming the attr_ops table
(N_ATTRS x D x D fp32 = 512 MB). Samples are grouped by attribute on the host,
groups are load-balanced across the 8 cores (snake deal by group size), and
each core streams only its own subset of operator matrices from HBM exactly
once, quantized on the host to fp8 e3m4 (TRN FP8_EXP3, 4 mantissa bits) with
a global x128 scale folded into the fp16 x vectors — 1 byte/elem halves the
HBM stream vs fp16 (rel err 1.39e-2 vs the f32 reference, under the 2e-2
gate; e4m3 fails at 3.2e-2). 63 matrices/core = 16.5 MB -> 46.2 us at the
358 GB/s per-core HBM cap; +xt/out traffic the HBM floor is ~48.2 us.

MODE="raw" (current, ~51.5 us/iter): hand-scheduled bass, no Tile
scheduling. Measured motivation: Tile attaches a sem update to every matmul
(they serialize at ~26 ns at the EVT_SEM register), inserts a full
drain+reset barrier between loop iterations (~4 us), and the PE+DMA streams
ended up only ~40% overlapped (66.5 us total vs 46 us DMA-alone). The raw
version:
  * credit/token semaphores whose wait targets live in per-engine registers
    (monotonic, iteration-invariant deltas) -> NO inter-iteration barrier;
    the 63-slot ops ring spans loop iterations and the act/out tail of
    iteration i overlaps the DMA stream of iteration i+1;
  * one sem inc per GROUP (then_inc on its last matmul; sound: PE
    completions are pc-ordered), one per-slot DMA-completion sem LANE
    (a single shared sem is UNSOUND with >1 DMA in flight: the 16 per-DMA
    increments come from 16 independent SDMA engines and interleave);
  * hybrid orientation: first 46 slots flip (A^T chunks stationary, fp8
    fast-weight-load, 16 matmuls/matrix, accumulating out^T in 4 PSUM
    banks), last NF=17 slots non-flip (x columns stationary, matrix rows
    moving, 4 matmuls/matrix into 4 dedicated PSUM banks cycled with
    eager per-slot DVE relu). Measured: a concurrent SDMA stream is slowed
    ~9 us by the flip-only PE instruction stream (1008 instr); the hybrid
    (756+68 instr, PE busy still < DMA) cuts that to ~3 us. All-non-flip
    is PE-bound (54 us) and slower. NB tile_position col-packing for the
    nf PSUM crashes the exec unit (NRT 101) — use whole banks only.
Sems are zeroed at kernel end so NEFF re-executions start clean; sem lanes
use nb=63 (= ng) so every lane has exactly one DMA/iteration and all lanes
share one wait-target register.

Known dead ends (measured, do not retry blindly):
  * pair=2 DMA batching (4 KB/partition runs, ~0.5-1 us upside): the first
    EXECUTION of the NEFF corrupts the first two non-flip slots (one stale
    qj chunk) even though the emitted waits/addresses are correct;
    re-executions are silently "correct" only because stale SBUF equals the
    previous execution's identical weights. Genuine read-before-land race,
    root cause unidentified -> pair stays 1.
  * sem_clear at kernel START (gpsimd + _nrt_pseudo_barrier or
    + all_engine_barrier): races the engine streams, rel err ~0.3-0.8.
  * ALIGN=2 layout: rel err 0.68 (latent 4-alignment assumption somewhere
    in the raw path).
  * tile_position col-packing for nf PSUM: NRT 101 exec-unit crash.

Previous Tile-based implementation (MODE="tile", ~64.6-66.5 us) is kept
for A/B; its ablations: fp16 streaming 108 us, fp8 DMA alone 47 us, PE
alone 39 us. The SPMD program is identical on all 8 cores; only per-core
tensors differ. Slot s has a fixed column capacity maxc[s] = max over
cores of that rank's group size, so the one program fits every core's
routing. Timing method (test.py): paired hardware-loop wall-delta,
(wall(R=2001) - wall(R=201)) / 1800.
"""

import numpy as np
import ml_dtypes

import concourse.tile as tile
from concourse import bacc, mybir
from concourse.bass_utils import run_bass_kernel_spmd

N_CORES = 8
D = 512               # embedding dim (hardcoded per problem spec)
QCH = D // 128        # contraction chunks of 128 partitions
# attr_ops stream is fp8 e3m4 (TRN FP8_EXP3): normals cover [0.25, 15.5], so
# scale A up by 128 (|A|max ~0.11 -> ~13.9) and fold 1/128 into x on the host.
A_SCALE = 128.0
E3M4 = ml_dtypes.float8_e3m4

# test.py hooks (ignored by the grading harness)
LAST_RESULTS = None   # BassKernelResults of the most recent run
TRACE = False
TRACE_CORES = None

PAIR = 1
# Slot column offsets aligned to 4 -> every matmul's f32 PSUM write starts
# 16B-cacheline-aligned and every fp16 xt read 8B-aligned (PSUM lines are
# 8B, SBUF lines 16B; misaligned partial-line PSUM writes measurably slow
# the matmul stream: align=4 beat align=1 by ~5us/iter in-process).
ALIGN = 4
_NC_CACHE = {}


def _build_nc(maxc, offs, ncol, ops_bufs=8, pair=1, sync_frac=(1, 1), reps=1,
              out_engine="scalar", staggered=False, relu_engine="scalar",
              xt_engine="scalar", ops_dt="f8e3", out_dt="f16",
              do_ops_dma=True, do_mm=True, do_act=True, do_out=True):
    """Build + compile the SPMD program.

    maxc[s]: column capacity of slot s; offs[s]: column offset of slot s;
    ncol: total columns (= offs[-1] + maxc[-1]).
    pair: matrices loaded per ops DMA (amortizes per-DMA fixed costs).
    sync_frac: (a, b) -> a of every b ops DMAs issue on sync, rest on scalar.
    reps: hardware-loop repetitions of the whole kernel (for timing).
    staggered: staggered-reset loop back-edge — wedges this device, keep False.
    """
    nm = len(maxc)
    nmp = -(-nm // pair) * pair  # nm rounded up to a multiple of pair
    ng = nmp // pair
    mdt = {"f8e3": mybir.dt.float8e3, "f8e4": mybir.dt.float8e4,
           "f16": mybir.dt.float16}[ops_dt]
    odt = {"f16": mybir.dt.float16, "f32": mybir.dt.float32}[out_dt]
    nc = bacc.Bacc("TRN2", target_bir_lowering=False, debug=False,
                   num_devices=N_CORES)
    # per-group layout [p, t, q, i]: each partition's data is one contiguous
    # pair*QCH*D-element run -> one big DMA descriptor per partition
    ops_dram = nc.dram_tensor("ops_t", [ng, 128, pair * QCH * D],
                              mdt, kind="ExternalInput").ap()
    xt_dram = nc.dram_tensor("xt", [128, QCH * ncol], mybir.dt.float16,
                             kind="ExternalInput").ap()
    out_dram = nc.dram_tensor("out", [ncol, D], odt,
                              kind="ExternalOutput").ap()

    with tile.TileContext(nc) as tc:
        with (
            tc.tile_pool(name="xt", bufs=1) as xt_pool,
            tc.tile_pool(name="ops", bufs=ops_bufs) as ops_pool,
            tc.tile_pool(name="ps", bufs=8, space="PSUM") as ps_pool,
            tc.tile_pool(name="o", bufs=4) as o_pool,
        ):
            def body():
                xt_sb = xt_pool.tile([128, QCH * ncol], mybir.dt.float16)
                getattr(nc, xt_engine).dma_start(xt_sb[:], xt_dram[:])
                if not do_ops_dma:
                    m0 = ops_pool.tile([128, pair * QCH * D], mdt, tag="m")
                    nc.sync.dma_start(m0[:], ops_dram[0])

                for g in range(ng):
                    if do_ops_dma:
                        m = ops_pool.tile([128, pair * QCH * D], mdt, tag="m")
                        issuer = nc.sync if g % sync_frac[1] < sync_frac[0] \
                            else nc.scalar
                        issuer.dma_start(m[:], ops_dram[g])
                    else:
                        m = m0
                    for t in range(pair):
                        s = g * pair + t
                        if s >= nm:
                            break
                        cw = maxc[s]
                        if not do_mm:
                            continue
                        ps = ps_pool.tile([cw, D], mybir.dt.float32, tag="ps")
                        for q in range(QCH):
                            lhsT = xt_sb[:, q * ncol + offs[s]:
                                         q * ncol + offs[s] + cw]
                            rhs = m[:, (t * QCH + q) * D:
                                    (t * QCH + q + 1) * D]
                            nc.tensor.matmul(ps[:], lhsT, rhs,
                                             start=(q == 0),
                                             stop=(q == QCH - 1))
                        if not do_act:
                            continue
                        o = o_pool.tile([cw, D], odt, tag="o")
                        if relu_engine == "vector":
                            nc.vector.tensor_scalar_max(o[:], ps[:], 0.0)
                        else:
                            nc.scalar.activation(
                                o[:], ps[:], mybir.ActivationFunctionType.Relu)
                        if not do_out:
                            continue
                        out_eng = getattr(nc, out_engine)
                        out_eng.dma_start(
                            out_dram[offs[s]:offs[s] + cw, :], o[:])

            if reps == 1:
                body()
            else:
                with tc.For_i(0, reps, 1,
                              hint_engines=(mybir.EngineType.PE,),
                              staggered_reset=staggered):
                    body()

    nc.compile()
    return nc


def _build_nc_flip(maxc, offs, ncol, ops_bufs=8, pair=1, sync_frac=(1, 1),
                   reps=1, out_engine="scalar", staggered=False,
                   relu_engine="vector", xt_engine="scalar", ops_dt="f8e3",
                   out_dt="f16", do_ops_dma=True, do_mm=True, do_act=True,
                   do_out=True, mm_src="real", mm_every=1, dma_split=False,
                   ops_engine=None, act_split=2, xt_split=True,
                   out_per_qi=True, mm_order="qi", mm_split=1,
                   ops_frac=1, prog_out=0):
    """Flipped orientation: A chunks are the stationary operand (fp8 weights
    -> fast weight load), x columns stream as the moving operand.

    Per slot s (one operator matrix A), for each output chunk qi and
    contraction chunk qj: ldweights(A^T[qj,qi] 128x128) + matmul over the
    slot's cw x-columns, accumulating out^T[qi*128:+128, cols(s)] in a PSUM
    tile [128, ncol] shared by all slots. One ReLU per qi over the full
    [128, ncol] bank, one contiguous output DMA of out^T.
    """
    nm = len(maxc)
    nmp = -(-nm // pair) * pair
    ng = nmp // pair
    mdt = {"f8e3": mybir.dt.float8e3, "f8e4": mybir.dt.float8e4,
           "f16": mybir.dt.float16}[ops_dt]
    odt = {"f16": mybir.dt.float16, "f32": mybir.dt.float32}[out_dt]
    nc = bacc.Bacc("TRN2", target_bir_lowering=False, debug=False,
                   num_devices=N_CORES)
    # ops_t[g, p, ((t*QCH+qj)*QCH+qi)*128 + i] = s*A_s[qi*128+i, qj*128+p]
    # ops_frac>1: timing-probe mode — stream only 1/ops_frac of the bytes
    # (results are garbage; used to measure DMA-vs-PE scaling).
    gsz = pair * QCH * QCH * 128 // ops_frac
    ops_dram = nc.dram_tensor("ops_t", [ng, 128, gsz],
                              mdt, kind="ExternalInput").ap()
    xt_dram = nc.dram_tensor("xt", [128, QCH * ncol], mybir.dt.float16,
                             kind="ExternalInput").ap()
    # out^T: out_dram[p, qi*ncol + c] = out[c, qi*128+p]
    out_dram = nc.dram_tensor("out", [128, QCH * ncol], odt,
                              kind="ExternalOutput").ap()

    with tile.TileContext(nc) as tc:
        with (
            tc.tile_pool(name="xt", bufs=1) as xt_pool,
            tc.tile_pool(name="ops", bufs=ops_bufs) as ops_pool,
            tc.tile_pool(name="m0p", bufs=1) as m0_pool,
            tc.tile_pool(name="ps", bufs=8, space="PSUM") as ps_pool,
            tc.tile_pool(name="o", bufs=2) as o_pool,
        ):
            def body():
                xt_sb = xt_pool.tile([128, QCH * ncol], mybir.dt.float16)
                if xt_split:
                    for qj in range(QCH):
                        getattr(nc, xt_engine).dma_start(
                            xt_sb[:, qj * ncol:(qj + 1) * ncol],
                            xt_dram[:, qj * ncol:(qj + 1) * ncol])
                else:
                    getattr(nc, xt_engine).dma_start(xt_sb[:], xt_dram[:])
                ps = [ps_pool.tile([128, ncol], mybir.dt.float32, tag="ps",
                                   name=f"ps{qi}")
                      for qi in range(QCH)]
                if not do_ops_dma or mm_src == "m0":
                    m0 = m0_pool.tile([128, gsz], mdt, tag="m0", bufs=1)
                    nc.sync.dma_start(m0[:], ops_dram[0])

                # progressive act/out: emit ReLU + out DMA for a column span
                # once the last slot covering it has been multiplied, instead
                # of serially after the whole stream.
                span_end = {}           # slot s -> (c0, c1) to flush after s
                o_prog = None
                if prog_out and do_act:
                    o_prog = o_pool.tile([128, QCH * ncol], odt, tag="o",
                                         bufs=1)
                    bounds = [nm * (j + 1) // prog_out - 1
                              for j in range(prog_out)]
                    c_prev = 0
                    for s_e in bounds:
                        c_hi = offs[s_e] + maxc[s_e]
                        span_end[s_e] = (c_prev, c_hi)
                        c_prev = c_hi

                def flush_span(c0, c1):
                    for qi in range(QCH):
                        dst = o_prog[:, qi * ncol + c0:qi * ncol + c1]
                        src = ps[qi][:, c0:c1]
                        if relu_engine == "vector":
                            nc.vector.tensor_scalar_max(dst, src, 0.0)
                        else:
                            nc.scalar.activation(
                                dst, src, mybir.ActivationFunctionType.Relu)
                        if do_out:
                            getattr(nc, out_engine).dma_start(
                                out_dram[:, qi * ncol + c0:qi * ncol + c1],
                                dst)

                for g in range(ng):
                    if do_ops_dma:
                        m = ops_pool.tile([128, gsz], mdt, tag="m")
                        if dma_split:
                            h = gsz // 2
                            nc.sync.dma_start(m[:, :h], ops_dram[g][:, :h])
                            nc.scalar.dma_start(m[:, h:], ops_dram[g][:, h:])
                        elif ops_engine is not None:
                            getattr(nc, ops_engine).dma_start(
                                m[:], ops_dram[g])
                        else:
                            issuer = nc.sync \
                                if g % sync_frac[1] < sync_frac[0] \
                                else nc.scalar
                            issuer.dma_start(m[:], ops_dram[g])
                        if mm_src == "m0":
                            m = m0
                    else:
                        m = m0
                    for t in range(pair):
                        s = g * pair + t
                        if s >= nm:
                            break
                        cw = maxc[s]
                        if not do_mm or s % mm_every:
                            continue
                        order = [(qi, qj) for qi in range(QCH)
                                 for qj in range(QCH)] \
                            if mm_order == "qi" else \
                            [(qi, qj) for qj in range(QCH)
                             for qi in range(QCH)]
                        for qi, qj in order:
                            ck = (((t * QCH + qj) * QCH + qi)
                                  % (gsz // 128)) * 128
                            lhsT = m[:, ck:ck + 128]
                            for h in range(mm_split):
                                a0 = offs[s] + cw * h // mm_split
                                a1 = offs[s] + cw * (h + 1) // mm_split
                                if a1 == a0:
                                    continue
                                rhs = xt_sb[:, qj * ncol + a0:
                                            qj * ncol + a1]
                                nc.tensor.matmul(
                                    ps[qi][:, a0:a1],
                                    lhsT, rhs, start=(qj == 0),
                                    stop=(qj == QCH - 1))
                if not do_act:
                    return
                o = o_pool.tile([128, QCH * ncol], odt, tag="o")
                for qi in range(QCH):
                    for h in range(act_split):
                        c0 = ncol * h // act_split
                        c1 = ncol * (h + 1) // act_split
                        dst = o[:, qi * ncol + c0:qi * ncol + c1]
                        src = ps[qi][:, c0:c1]
                        if relu_engine == "vector":
                            nc.vector.tensor_scalar_max(dst, src, 0.0)
                        else:
                            nc.scalar.activation(
                                dst, src, mybir.ActivationFunctionType.Relu)
                    if do_out and out_per_qi:
                        getattr(nc, out_engine).dma_start(
                            out_dram[:, qi * ncol:(qi + 1) * ncol],
                            o[:, qi * ncol:(qi + 1) * ncol])
                if do_out and not out_per_qi:
                    getattr(nc, out_engine).dma_start(out_dram[:], o[:])

            if reps == 1:
                body()
            else:
                with tc.For_i(0, reps, 1,
                              hint_engines=(mybir.EngineType.PE,),
                              staggered_reset=staggered):
                    body()

    nc.compile()
    return nc


NF = 17   # tail slots computed in non-flip orientation (hybrid)


def _build_raw(maxc, offs, ncol, reps=1, nb=63, do_ops_dma=True, do_mm=True,
               do_act=True, do_out=True, xt_chunks=1, use_regs=True,
               do_clear=False, ops_engine="sync", slot_stride=1,
               nf_probe=0, nf=None, pair=1):
    """Raw-bass (no Tile scheduling) flip-orientation kernel.

    Motivation (measured): Tile attaches a semaphore update to every matmul
    (1008/iter) and a full engine drain+reset between loop iterations; the
    per-instruction sem updates serialize at ~26ns each at the EVT_SEM
    register and the PE+DMA streams end up only ~40% overlapped (base 66.5us
    vs 46us DMA-alone line rate). This version hand-manages semaphores:
    one inc per ops-DMA (hardware, +16), one inc per GROUP on PE (last
    matmul's then_inc, sound because PE completions are pc-ordered), and a
    credit scheme whose wait targets are carried in per-engine registers so
    the hardware loop needs NO inter-iteration barrier: the ops-DMA ring
    spans iteration boundaries and the act/out tail of iteration i overlaps
    the DMA stream of iteration i+1.

    Engine program (per iteration):
      sync  : ng x [credit wait on pe_sem; dma_start ops -> ring (+16 dma_sem)]
      tensor: wait xt_sem; wait act_sem (PSUM free);
              ng x [wait dma_sem; 16 matmuls; last +1 pe_sem]
      scalar: wait pe_sem (xt free); dma xt (+16 xt_sem);
              4 x [wait act_sem; dma out qi (+16 out_sem)]
      vector: wait pe_sem (all groups); wait out_sem (o free);
              8 x relu chunk (+1 act_sem each)
    """
    nm = len(maxc)
    ng = nm
    if nf is None:
        nf = 0 if nf_probe else NF
    nf = min(nf, ng)
    # HEAD placement: slots 0..nf-1 are non-flip (measured ~0.9us faster
    # than tail placement); flip slots own columns [c_nf, ncol)
    assert all(maxc[s] <= 32 for s in range(nf))
    c_nf = offs[nf] if nf else 0                 # first flip column
    noffs = [0]                                  # out_nf row offsets
    for s in range(nf):
        noffs.append(noffs[-1] + maxc[s])
    n_out = 4 + nf                               # out DMAs per iteration
    n_act = 8 + nf                               # act_sem incs per iteration

    nc = bacc.Bacc("TRN2", target_bir_lowering=False, debug=False,
                   num_devices=N_CORES)
    gsz = QCH * QCH * 128
    # pair>1: `pair` matrices per DMA (bigger per-partition runs amortize
    # SDMA packet overhead); the last DMA is truncated so padding matrices
    # are never transferred. Forces whole-iteration ring (nb = ng).
    ng_dma = -(-ng // pair)
    gszp = pair * gsz
    if pair > 1:
        nb = ng
    ops_dram = nc.dram_tensor("ops_t", [ng_dma, 128, gszp],
                              mybir.dt.float8e3,
                              kind="ExternalInput").ap()
    xt_dram = nc.dram_tensor("xt", [128, QCH * ncol], mybir.dt.float16,
                             kind="ExternalInput").ap()
    out_dram = nc.dram_tensor("out", [128, QCH * ncol], mybir.dt.float16,
                              kind="ExternalOutput").ap()
    out_nf_dram = nc.dram_tensor(
        "out_nf", [max(1, noffs[-1]), D], mybir.dt.float16,
        kind="ExternalOutput").ap()

    xt_sb = nc.alloc_sbuf_tensor("xt_sb", [128, QCH * ncol],
                                 mybir.dt.float16).ap()
    o_sb = nc.alloc_sbuf_tensor("o_sb", [128, QCH * ncol],
                                mybir.dt.float16).ap()
    o_nf = nc.alloc_sbuf_tensor("o_nf", [128, 4 * D], mybir.dt.float16).ap()
    mring = nc.alloc_sbuf_tensor("mring", [128, ng_dma * gszp],
                                 mybir.dt.float8e3).ap()
    ps = [nc.alloc_psum_tensor(f"ps{qi}", [128, ncol], mybir.dt.float32).ap()
          for qi in range(QCH)]
    # 4 PSUM banks for the non-flip tail, cycled with reuse distance 4
    # (no tile_position partition packing: that path crashes the exec unit)
    ps_nf = [nc.alloc_psum_tensor(f"ps_nf{b}", [128, D],
                                  mybir.dt.float32).ap()
             for b in range(4)] if (nf or nf_probe) else None

    # dma completion sems: one LANE per ring slot. A single shared sem is
    # unsound with >1 DMA in flight: the 16 per-DMA increments come from 16
    # independent SDMA engines, so increments of DMA k+1 can stand in for
    # laggards of DMA k and a threshold wait passes before k fully lands
    # (observed: corruption starting exactly where PE catches the stream).
    # With lane=slot and ring credits bounding in-flight <= nb, each lane
    # has at most one DMA outstanding -> its count is unambiguous.
    # nb must divide ng so lane assignment is iteration-invariant.
    assert ng % nb == 0, (ng, nb)
    n_lanes = ng_dma if pair > 1 else nb
    dma_sems = [nc.alloc_semaphore(f"dma_sem{l}") for l in range(n_lanes)]
    xt_sem = nc.alloc_semaphore("xt_sem")
    pe_sem = nc.alloc_semaphore("pe_sem")
    act_sem = nc.alloc_semaphore("act_sem")
    out_sem = nc.alloc_semaphore("out_sem")
    sems = dma_sems + [xt_sem, pe_sem, act_sem, out_sem]

    # NB: do NOT clear sems at kernel start — both a gpsimd sem_clear +
    # _nrt_pseudo_barrier and a sem_clear + all_engine_barrier preamble
    # measurably RACE the engine streams here (verified: rel err jumps to
    # ~0.3-0.8). Instead sems are zeroed at kernel END (below), and the
    # program assumes zeroed sems at entry.
    if do_clear:
        for s in sems:
            nc.gpsimd.sem_clear(s)
        nc._nrt_pseudo_barrier()

    # ring credits: pre-seed nb so the first nb ops DMAs don't wait
    nc.tensor.sem_inc(pe_sem, nb)

    class _Ctr:
        """Cumulative wait target: a per-engine register, or (debug,
        reps==1 only) a compile-time constant."""
        def __init__(self, eng, name, init):
            self.eng, self.val = eng, init
            if use_regs:
                self.reg = eng.alloc_register(name)
                eng.reg_mov(self.reg, init)
        def add(self, d):
            self.val += d
            if use_regs:
                self.eng.reg_add(self.reg, self.reg, d)
        def wait(self, sem):
            self.eng.wait_ge(sem, self.reg if use_regs else self.val)
        def bump_to(self, target):
            # target sequences are iteration-invariant in their deltas,
            # so the emitted reg_adds replay correctly every loop pass
            assert target >= self.val, (target, self.val)
            if target > self.val:
                self.add(target - self.val)

    if not use_regs:
        assert reps == 1
    r_credit = _Ctr(nc.sync, "r_credit", 0)
    if nb == ng or pair > 1:
        # one DMA per lane per iteration -> every lane's target is the
        # same 16*(iter+1): share a single counter, bumped once per iter.
        r_dma_iter = _Ctr(nc.tensor, "r_dma", 16)
        r_dma = None
    else:
        r_dma = [_Ctr(nc.tensor, f"r_dma{l}", 0) for l in range(nb)]
    r_xt = _Ctr(nc.tensor, "r_xt", 16)
    r_actpe = _Ctr(nc.tensor, "r_actpe", 0)
    r_xtfree = _Ctr(nc.scalar, "r_xtfree", nb)
    r_act = _Ctr(nc.scalar, "r_act", 0)
    r_pe = _Ctr(nc.vector, "r_pe", nb)
    r_out = _Ctr(nc.vector, "r_out", 0)

    act_chunks = []   # (qi, c0, c1) in DVE issue order, 2 per qi; flip
    wf = ncol - c_nf    # columns only — nf head columns go via out_nf
    for qi in range(QCH):
        for h in range(2):
            act_chunks.append((qi, c_nf + wf * h // 2,
                               c_nf + wf * (h + 1) // 2))

    assert np.gcd(slot_stride, nb) == 1
    slot_of = lambda s: (s % nb) * slot_stride % nb

    with nc.Fori(0, reps, 1):
        # --- sync: ops stream into the ring ---
        if do_ops_dma and pair > 1:
            for d in range(ng_dma):
                # slot d's buffers hold groups [d*pair, min((d+1)*pair, ng))
                r_credit.bump_to(min(pair * (d + 1), ng))
                r_credit.wait(pe_sem)
                w = (min(pair * (d + 1), ng) - pair * d) * gsz
                nc.sync.dma_start(
                    mring[:, d * gszp:d * gszp + w], ops_dram[d][:, :w]
                ).then_inc(dma_sems[d], 16)
        elif do_ops_dma:
            for g in range(ng):
                r_credit.add(1)
                r_credit.wait(pe_sem)
                slot = slot_of(g)
                if ops_engine == "alt":
                    eng = nc.sync if g % 2 == 0 else nc.scalar
                else:
                    eng = getattr(nc, ops_engine)
                eng.dma_start(
                    mring[:, slot * gsz:(slot + 1) * gsz], ops_dram[g]
                ).then_inc(dma_sems[slot], 16)

        # --- scalar: xt in, outs out ---
        r_xtfree.wait(pe_sem)
        if xt_chunks == 1:
            nc.scalar.dma_start(xt_sb, xt_dram).then_inc(xt_sem, 16)
        else:
            w = QCH * ncol
            for c in range(xt_chunks):
                a, b = w * c // xt_chunks, w * (c + 1) // xt_chunks
                nc.scalar.dma_start(xt_sb[:, a:b], xt_dram[:, a:b]
                                    ).then_inc(xt_sem, 16)
        if do_out and do_act:
            for k in range(nf):
                u = k % 4
                cw = maxc[k]
                r_act.bump_to(k + 1)   # act inc of nf relu k (iter-0 vals)
                r_act.wait(act_sem)
                nc.scalar.dma_start(
                    out_nf_dram[noffs[k]:noffs[k] + cw, :],
                    o_nf[0:cw, u * D:(u + 1) * D]
                ).then_inc(out_sem, 16)
            for qi in range(QCH):
                r_act.bump_to(nf + 2 * (qi + 1))  # flip chunks come last
                r_act.wait(act_sem)
                nc.scalar.dma_start(
                    out_dram[:, qi * ncol + c_nf:(qi + 1) * ncol],
                    o_sb[:, qi * ncol + c_nf:(qi + 1) * ncol]
                ).then_inc(out_sem, 16)
        r_xtfree.add(ng)

        # --- tensor: the matmul stream ---
        r_xt.wait(xt_sem)
        r_actpe.wait(act_sem)
        for s in range(ng):
            if do_mm:
                if pair > 1:
                    d, t = divmod(s, pair)
                    if do_ops_dma and t == 0:
                        r_dma_iter.wait(dma_sems[d])
                    mbase = d * gszp + t * gsz
                else:
                    lane = slot_of(s)
                    if nb != ng:
                        r_dma[lane].add(16)
                    if do_ops_dma:
                        (r_dma_iter if nb == ng else r_dma[lane]).wait(
                            dma_sems[lane])
                    mbase = lane * gsz
                slot = None
                cw = maxc[s]
                last = None
                k = s if s < nf else -1
                if not nf:
                    k = -1
                if nf_probe < 0 and (s % 4 == 1 if nf_probe == -1
                                     else s < -nf_probe):
                    # TIMING PROBE ONLY (wrong math): non-flip-shaped slots
                    # placed mid-stream (-1: interleaved) or at the HEAD
                    # (-n: first n slots)
                    for qj in range(QCH):
                        lhsT = xt_sb[:, qj * ncol + offs[s]:
                                     qj * ncol + offs[s] + cw]
                        rhs = mring[:, mbase + qj * (QCH * 128):
                                    mbase + (qj + 1) * (QCH * 128)]
                        last = nc.tensor.matmul(
                            ps_nf[(s // 4) % 4][0:cw, :], lhsT, rhs,
                            start=(qj == 0), stop=(qj == QCH - 1))
                elif nf_probe > 0 and s >= ng - nf_probe:
                    # TIMING PROBE ONLY (wrong math): non-flip shape —
                    # 4 matmuls/matrix, x columns stationary, matrix moving.
                    for qj in range(QCH):
                        lhsT = xt_sb[:, qj * ncol + offs[s]:
                                     qj * ncol + offs[s] + cw]
                        rhs = mring[:, mbase + qj * (QCH * 128):
                                    mbase + (qj + 1) * (QCH * 128)]
                        last = nc.tensor.matmul(
                            ps_nf[0][0:cw, :], lhsT, rhs,
                            start=(qj == 0), stop=(qj == QCH - 1))
                elif k >= 0:
                    # non-flip: x columns stationary, matrix rows moving.
                    # PSUM: 4 dedicated banks, reused every 4 nf slots;
                    # a bank is freed by the eager DVE relu of slot k-4
                    # (act inc #(k-4+1) of this iteration: nf relus come
                    # FIRST in the act order under head placement).
                    b = k % 4
                    if k >= 4:
                        r_actpe.bump_to(k - 3)   # iter-0 target (k-4)+1
                        r_actpe.wait(act_sem)
                    for qj in range(QCH):
                        lhsT = xt_sb[:, qj * ncol + offs[s]:
                                     qj * ncol + offs[s] + cw]
                        rhs = mring[:, mbase + qj * D:
                                    mbase + (qj + 1) * D]
                        last = nc.tensor.matmul(
                            ps_nf[b][0:cw, :], lhsT, rhs,
                            start=(qj == 0), stop=(qj == QCH - 1))
                else:
                    for qi in range(QCH):
                        for qj in range(QCH):
                            ck = mbase + (qj * QCH + qi) * 128
                            lhsT = mring[:, ck:ck + 128]
                            rhs = xt_sb[:, qj * ncol + offs[s]:
                                        qj * ncol + offs[s] + cw]
                            last = nc.tensor.matmul(
                                ps[qi][:, offs[s]:offs[s] + cw], lhsT, rhs,
                                start=(qj == 0), stop=(qj == QCH - 1))
                last.then_inc(pe_sem, 1)
            else:
                nc.tensor.sem_inc(pe_sem, 1)
        r_xt.add(16)
        r_actpe.bump_to(n_act)
        if nb == ng:
            r_dma_iter.add(16)

        # --- vector: relu PSUM -> SBUF (nf head slots eagerly, then
        # the flip chunks once all flip slots are done) ---
        if do_act:
            r_out.wait(out_sem)          # prev iteration's outs landed
            for k in range(nf):
                u = k % 4
                cw = maxc[k]
                r_pe.bump_to(nb + k + 1)     # nf slot k done
                r_pe.wait(pe_sem)
                if k >= 4:
                    # o_nf column position reused: out DMA of slot k-4
                    # (out #(k-4+1) of this iteration) must have landed
                    r_out.bump_to(16 * (k - 3))
                    r_out.wait(out_sem)
                nc.vector.tensor_scalar_max(
                    o_nf[0:cw, u * D:(u + 1) * D],
                    ps_nf[u][0:cw, :], 0.0
                ).then_inc(act_sem, 1)
            r_pe.bump_to(nb + ng)        # all flip slots done
            r_pe.wait(pe_sem)
            for qi, c0, c1 in act_chunks:
                nc.vector.tensor_scalar_max(
                    o_sb[:, qi * ncol + c0:qi * ncol + c1],
                    ps[qi][:, c0:c1], 0.0
                ).then_inc(act_sem, 1)
            r_out.bump_to(16 * n_out)
        else:
            r_pe.add(ng)
            for _ in range(n_act):
                nc.vector.sem_inc(act_sem, 1)

    # quiesce: last iteration's out DMAs must have landed, then zero the
    # sems so a re-execution of this NEFF starts from clean state (waits
    # use absolute monotonic targets).
    if do_out and do_act:
        nc.scalar.wait_ge(out_sem, 64 * reps)
    nc.all_engine_barrier()
    for s in sems:
        nc.sync.sem_clear(s)
    nc.all_engine_barrier()
    nc.compile()
    return nc


def _route(attrs):
    """Group sample indices by attribute, chunk to <=128, snake-balance
    across cores. Returns per-core slot lists of (attr_id, idx_array),
    each list sorted by descending group size."""
    order = np.argsort(attrs, kind="stable")
    sorted_attrs = attrs[order]
    uniq, starts, counts = np.unique(sorted_attrs, return_index=True,
                                     return_counts=True)
    chunks = []
    for a, st, c in zip(uniq, starts, counts):
        idx = order[st:st + c]
        for o in range(0, c, 128):
            chunks.append((int(a), idx[o:o + 128]))
    chunks.sort(key=lambda t: -len(t[1]))
    per_core = [[] for _ in range(N_CORES)]
    for i, ch in enumerate(chunks):
        r, pos = divmod(i, N_CORES)
        k = pos if r % 2 == 0 else N_CORES - 1 - pos
        per_core[k].append(ch)
    return per_core


def _layout(per_core, align=1):
    """Per-slot-rank column capacity/offset shared by all cores.

    align: round capacities up so every slot's column offset is a multiple
    of `align` (align=2 makes f32 PSUM writes 8B-cacheline-aligned).
    """
    nm = max(1, max(len(s) for s in per_core))
    maxc = [1] * nm
    for slots in per_core:
        for s, (_, idx) in enumerate(slots):
            maxc[s] = max(maxc[s], len(idx))
    maxc = [-(-c // align) * align for c in maxc]
    offs = [0] * nm
    for s in range(1, nm):
        offs[s] = offs[s - 1] + maxc[s - 1]
    ncol = offs[-1] + maxc[-1]
    return nm, maxc, offs, ncol


def _prepare(attrs, objs, attr_ops, obj_emb, orient="flip", pair=None,
             align=None):
    """Route + build per-core device input maps."""
    if pair is None:
        pair = PAIR
    if align is None:
        align = ALIGN
    per_core = _route(attrs)
    nm, maxc, offs, ncol = _layout(per_core, align=align)
    nmp = -(-nm // pair) * pair

    rep = obj_emb[objs] * np.float32(1.0 / A_SCALE)  # [B, D], 1/s folded in
    ng = nmp // pair
    # raw hybrid: the last NF slots are computed non-flip on-device and
    # need the row-major (moving-operand) layout instead
    nf_eff = NF if (MODE == "raw" and orient == "flip") else 0
    in_maps = []
    for k in range(N_CORES):
        slots = per_core[k]
        ops_t = np.zeros((ng, 128, pair, QCH, QCH, 128), E3M4)
        r = np.zeros((ncol, D), np.float32)
        for s, (a, idx) in enumerate(slots):
            g, t = divmod(s, pair)
            at = np.clip(attr_ops[a].T * A_SCALE, -15.5, 15.5).astype(E3M4)
            if orient == "flip" and s >= nf_eff:
                # ops_t[g, p, t, qj, qi, i] = s*A[qi*128+i, qj*128+p]
                ops_t[g, :, t] = at.reshape(QCH, 128, QCH, 128).transpose(
                    1, 0, 2, 3)
            else:
                # ops_t[g, p, t, q, i] = s*A[i, q*128+p]
                ops_t[g, :, t] = at.reshape(QCH, 128, D).transpose(
                    1, 0, 2).reshape(128, QCH, QCH, 128)
            r[offs[s]:offs[s] + len(idx)] = rep[idx]
        # xt[p, q*ncol + c] = r[c, q*128 + p]
        xt = np.ascontiguousarray(r.reshape(ncol, QCH, 128).transpose(
            2, 1, 0).astype(np.float16)).reshape(128, -1)
        in_maps.append({"ops_t": ops_t.reshape(ng, 128, pair * QCH * D),
                        "xt": xt})
    return per_core, (nm, tuple(maxc), tuple(offs), ncol), in_maps


ORIENT = "flip"
MODE = "raw"      # "raw" (hand-managed sems) or "tile"


def _builder(reps=1, **kw):
    if MODE == "raw":
        def build(maxc, offs, ncol, **kw2):
            return _build_raw(list(maxc), list(offs), ncol, reps=reps,
                              **{**kw, **kw2})
    else:
        def build(maxc, offs, ncol, **kw2):
            b = _build_nc_flip if ORIENT == "flip" else _build_nc
            return b(list(maxc), list(offs), ncol, reps=reps, pair=PAIR,
                     **{**kw, **kw2})
    return build


def build_timing(maxc, offs, ncol, reps):
    """test.py hook: build the looped variant for wall-delta timing."""
    return _builder(reps=reps)(maxc, offs, ncol)


def kernel(attrs, objs, attr_ops, obj_emb):
    global LAST_RESULTS
    attrs = np.asarray(attrs)
    objs = np.asarray(objs)
    attr_ops = np.asarray(attr_ops, dtype=np.float32)
    obj_emb = np.asarray(obj_emb, dtype=np.float32)
    B = attrs.shape[0]
    d = obj_emb.shape[1]
    assert d == D and attr_ops.shape[1:] == (D, D)

    per_core, (nm, maxc, offs, ncol), in_maps = _prepare(
        attrs, objs, attr_ops, obj_emb, orient=ORIENT)

    nc = _NC_CACHE.get((MODE, ORIENT, maxc))
    if nc is None:
        nc = _NC_CACHE[(MODE, ORIENT, maxc)] = _builder()(maxc, offs, ncol)

    res = run_bass_kernel_spmd(nc, in_maps, core_ids=list(range(N_CORES)),
                               trace=TRACE, trace_cores=TRACE_CORES)
    LAST_RESULTS = res

    nf_eff = NF if (MODE == "raw" and ORIENT == "flip") else 0
    noffs = [0]
    for s in range(nf_eff):
        noffs.append(noffs[-1] + maxc[s])
    out = np.zeros((B, d), np.float32)
    for k in range(N_CORES):
        out_k = res.results[k]["out"].astype(np.float32)
        if ORIENT == "flip":
            out_k = out_k.reshape(128, QCH, ncol).transpose(2, 1, 0).reshape(
                ncol, D)
        for s, (a, idx) in enumerate(per_core[k]):
            if s < nf_eff:
                out_nf = res.results[k]["out_nf"].astype(np.float32)
                out[idx] = out_nf[noffs[s]:noffs[s] + len(idx)]
            else:
                out[idx] = out_k[offs[s]:offs[s] + len(idx)]
    return out



# revision 62
# speedup vs baseline: 1.0195x; 1.0026x over previous
"""Trainium2 Bass kernel for nn_AttributeOperator (MoE-style routing).

Computes out[b] = relu(attr_ops[attrs[b]] @ obj_emb[objs[b]]) for b in [0, B).

Strategy (expert-parallel): the dominant cost is streaming the attr_ops table
(N_ATTRS x D x D fp32 = 512 MB). Samples are grouped by attribute on the host,
groups are load-balanced across the 8 cores (snake deal by group size), and
each core streams only its own subset of operator matrices from HBM exactly
once, quantized on the host to fp8 e3m4 (TRN FP8_EXP3, 4 mantissa bits) with
a global x128 scale folded into the fp16 x vectors — 1 byte/elem halves the
HBM stream vs fp16 (rel err 1.39e-2 vs the f32 reference, under the 2e-2
gate; e4m3 fails at 3.2e-2). 63 matrices/core = 16.5 MB -> 46.2 us at the
358 GB/s per-core HBM cap; +xt/out traffic the HBM floor is ~48.2 us.

MODE="raw" (current, ~51.5 us/iter): hand-scheduled bass, no Tile
scheduling. Measured motivation: Tile attaches a sem update to every matmul
(they serialize at ~26 ns at the EVT_SEM register), inserts a full
drain+reset barrier between loop iterations (~4 us), and the PE+DMA streams
ended up only ~40% overlapped (66.5 us total vs 46 us DMA-alone). The raw
version:
  * credit/token semaphores whose wait targets live in per-engine registers
    (monotonic, iteration-invariant deltas) -> NO inter-iteration barrier;
    the 63-slot ops ring spans loop iterations and the act/out tail of
    iteration i overlaps the DMA stream of iteration i+1;
  * one sem inc per GROUP (then_inc on its last matmul; sound: PE
    completions are pc-ordered), one per-slot DMA-completion sem LANE
    (a single shared sem is UNSOUND with >1 DMA in flight: the 16 per-DMA
    increments come from 16 independent SDMA engines and interleave);
  * hybrid orientation: FIRST NF=17 slots non-flip (x columns stationary,
    matrix rows moving, 4 matmuls/matrix into 4 dedicated PSUM banks
    cycled with eager per-slot DVE relu), remaining 46 slots flip (A^T
    chunks stationary, fp8 fast-weight-load, 16 matmuls/matrix,
    accumulating out^T in 4 PSUM banks). Head placement measured ~= tail,
    both beat interleaved (+7us) by far. Measured: a concurrent SDMA stream is slowed
    ~9 us by the flip-only PE instruction stream (1008 instr); the hybrid
    (756+68 instr, PE busy still < DMA) cuts that to ~3 us. All-non-flip
    is PE-bound (54 us) and slower. NB tile_position col-packing for the
    nf PSUM crashes the exec unit (NRT 101) — use whole banks only.
Sems are zeroed at kernel end so NEFF re-executions start clean; sem lanes
use nb=63 (= ng) so every lane has exactly one DMA/iteration and all lanes
share one wait-target register.

Known dead ends (measured, do not retry blindly):
  * pair=2 DMA batching (4 KB/partition runs, ~0.5-1 us upside): the first
    EXECUTION of the NEFF corrupts the first two non-flip slots (one stale
    qj chunk) even though the emitted waits/addresses are correct;
    re-executions are silently "correct" only because stale SBUF equals the
    previous execution's identical weights. Genuine read-before-land race,
    root cause unidentified -> pair stays 1.
  * sem_clear at kernel START (gpsimd + _nrt_pseudo_barrier or
    + all_engine_barrier): races the engine streams, rel err ~0.3-0.8.
  * ALIGN=2 layout: rel err 0.68 (latent 4-alignment assumption somewhere
    in the raw path).
  * tile_position col-packing for nf PSUM: NRT 101 exec-unit crash.

Previous Tile-based implementation (MODE="tile", ~64.6-66.5 us) is kept
for A/B; its ablations: fp16 streaming 108 us, fp8 DMA alone 47 us, PE
alone 39 us. The SPMD program is identical on all 8 cores; only per-core
tensors differ. Slot s has a fixed column capacity maxc[s] = max over
cores of that rank's group size, so the one program fits every core's
routing. Timing method (test.py): paired hardware-loop wall-delta,
(wall(R=2001) - wall(R=201)) / 1800.
"""

import numpy as np
import ml_dtypes

import concourse.tile as tile
from concourse import bacc, mybir
from concourse.bass_utils import run_bass_kernel_spmd

N_CORES = 8
D = 512               # embedding dim (hardcoded per problem spec)
QCH = D // 128        # contraction chunks of 128 partitions
# attr_ops stream is fp8 e3m4 (TRN FP8_EXP3): normals cover [0.25, 15.5], so
# scale A up by 128 (|A|max ~0.11 -> ~13.9) and fold 1/128 into x on the host.
A_SCALE = 128.0
E3M4 = ml_dtypes.float8_e3m4

# test.py hooks (ignored by the grading harness)
LAST_RESULTS = None   # BassKernelResults of the most recent run
TRACE = False
TRACE_CORES = None

PAIR = 1
# Slot column offsets aligned to 4 -> every matmul's f32 PSUM write starts
# 16B-cacheline-aligned and every fp16 xt read 8B-aligned (PSUM lines are
# 8B, SBUF lines 16B; misaligned partial-line PSUM writes measurably slow
# the matmul stream: align=4 beat align=1 by ~5us/iter in-process).
ALIGN = 4
_NC_CACHE = {}


def _build_nc(maxc, offs, ncol, ops_bufs=8, pair=1, sync_frac=(1, 1), reps=1,
              out_engine="scalar", staggered=False, relu_engine="scalar",
              xt_engine="scalar", ops_dt="f8e3", out_dt="f16",
              do_ops_dma=True, do_mm=True, do_act=True, do_out=True):
    """Build + compile the SPMD program.

    maxc[s]: column capacity of slot s; offs[s]: column offset of slot s;
    ncol: total columns (= offs[-1] + maxc[-1]).
    pair: matrices loaded per ops DMA (amortizes per-DMA fixed costs).
    sync_frac: (a, b) -> a of every b ops DMAs issue on sync, rest on scalar.
    reps: hardware-loop repetitions of the whole kernel (for timing).
    staggered: staggered-reset loop back-edge — wedges this device, keep False.
    """
    nm = len(maxc)
    nmp = -(-nm // pair) * pair  # nm rounded up to a multiple of pair
    ng = nmp // pair
    mdt = {"f8e3": mybir.dt.float8e3, "f8e4": mybir.dt.float8e4,
           "f16": mybir.dt.float16}[ops_dt]
    odt = {"f16": mybir.dt.float16, "f32": mybir.dt.float32}[out_dt]
    nc = bacc.Bacc("TRN2", target_bir_lowering=False, debug=False,
                   num_devices=N_CORES)
    # per-group layout [p, t, q, i]: each partition's data is one contiguous
    # pair*QCH*D-element run -> one big DMA descriptor per partition
    ops_dram = nc.dram_tensor("ops_t", [ng, 128, pair * QCH * D],
                              mdt, kind="ExternalInput").ap()
    xt_dram = nc.dram_tensor("xt", [128, QCH * ncol], mybir.dt.float16,
                             kind="ExternalInput").ap()
    out_dram = nc.dram_tensor("out", [ncol, D], odt,
                              kind="ExternalOutput").ap()

    with tile.TileContext(nc) as tc:
        with (
            tc.tile_pool(name="xt", bufs=1) as xt_pool,
            tc.tile_pool(name="ops", bufs=ops_bufs) as ops_pool,
            tc.tile_pool(name="ps", bufs=8, space="PSUM") as ps_pool,
            tc.tile_pool(name="o", bufs=4) as o_pool,
        ):
            def body():
                xt_sb = xt_pool.tile([128, QCH * ncol], mybir.dt.float16)
                getattr(nc, xt_engine).dma_start(xt_sb[:], xt_dram[:])
                if not do_ops_dma:
                    m0 = ops_pool.tile([128, pair * QCH * D], mdt, tag="m")
                    nc.sync.dma_start(m0[:], ops_dram[0])

                for g in range(ng):
                    if do_ops_dma:
                        m = ops_pool.tile([128, pair * QCH * D], mdt, tag="m")
                        issuer = nc.sync if g % sync_frac[1] < sync_frac[0] \
                            else nc.scalar
                        issuer.dma_start(m[:], ops_dram[g])
                    else:
                        m = m0
                    for t in range(pair):
                        s = g * pair + t
                        if s >= nm:
                            break
                        cw = maxc[s]
                        if not do_mm:
                            continue
                        ps = ps_pool.tile([cw, D], mybir.dt.float32, tag="ps")
                        for q in range(QCH):
                            lhsT = xt_sb[:, q * ncol + offs[s]:
                                         q * ncol + offs[s] + cw]
                            rhs = m[:, (t * QCH + q) * D:
                                    (t * QCH + q + 1) * D]
                            nc.tensor.matmul(ps[:], lhsT, rhs,
                                             start=(q == 0),
                                             stop=(q == QCH - 1))
                        if not do_act:
                            continue
                        o = o_pool.tile([cw, D], odt, tag="o")
                        if relu_engine == "vector":
                            nc.vector.tensor_scalar_max(o[:], ps[:], 0.0)
                        else:
                            nc.scalar.activation(
                                o[:], ps[:], mybir.ActivationFunctionType.Relu)
                        if not do_out:
                            continue
                        out_eng = getattr(nc, out_engine)
                        out_eng.dma_start(
                            out_dram[offs[s]:offs[s] + cw, :], o[:])

            if reps == 1:
                body()
            else:
                with tc.For_i(0, reps, 1,
                              hint_engines=(mybir.EngineType.PE,),
                              staggered_reset=staggered):
                    body()

    nc.compile()
    return nc


def _build_nc_flip(maxc, offs, ncol, ops_bufs=8, pair=1, sync_frac=(1, 1),
                   reps=1, out_engine="scalar", staggered=False,
                   relu_engine="vector", xt_engine="scalar", ops_dt="f8e3",
                   out_dt="f16", do_ops_dma=True, do_mm=True, do_act=True,
                   do_out=True, mm_src="real", mm_every=1, dma_split=False,
                   ops_engine=None, act_split=2, xt_split=True,
                   out_per_qi=True, mm_order="qi", mm_split=1,
                   ops_frac=1, prog_out=0):
    """Flipped orientation: A chunks are the stationary operand (fp8 weights
    -> fast weight load), x columns stream as the moving operand.

    Per slot s (one operator matrix A), for each output chunk qi and
    contraction chunk qj: ldweights(A^T[qj,qi] 128x128) + matmul over the
    slot's cw x-columns, accumulating out^T[qi*128:+128, cols(s)] in a PSUM
    tile [128, ncol] shared by all slots. One ReLU per qi over the full
    [128, ncol] bank, one contiguous output DMA of out^T.
    """
    nm = len(maxc)
    nmp = -(-nm // pair) * pair
    ng = nmp // pair
    mdt = {"f8e3": mybir.dt.float8e3, "f8e4": mybir.dt.float8e4,
           "f16": mybir.dt.float16}[ops_dt]
    odt = {"f16": mybir.dt.float16, "f32": mybir.dt.float32}[out_dt]
    nc = bacc.Bacc("TRN2", target_bir_lowering=False, debug=False,
                   num_devices=N_CORES)
    # ops_t[g, p, ((t*QCH+qj)*QCH+qi)*128 + i] = s*A_s[qi*128+i, qj*128+p]
    # ops_frac>1: timing-probe mode — stream only 1/ops_frac of the bytes
    # (results are garbage; used to measure DMA-vs-PE scaling).
    gsz = pair * QCH * QCH * 128 // ops_frac
    ops_dram = nc.dram_tensor("ops_t", [ng, 128, gsz],
                              mdt, kind="ExternalInput").ap()
    xt_dram = nc.dram_tensor("xt", [128, QCH * ncol], mybir.dt.float16,
                             kind="ExternalInput").ap()
    # out^T: out_dram[p, qi*ncol + c] = out[c, qi*128+p]
    out_dram = nc.dram_tensor("out", [128, QCH * ncol], odt,
                              kind="ExternalOutput").ap()

    with tile.TileContext(nc) as tc:
        with (
            tc.tile_pool(name="xt", bufs=1) as xt_pool,
            tc.tile_pool(name="ops", bufs=ops_bufs) as ops_pool,
            tc.tile_pool(name="m0p", bufs=1) as m0_pool,
            tc.tile_pool(name="ps", bufs=8, space="PSUM") as ps_pool,
            tc.tile_pool(name="o", bufs=2) as o_pool,
        ):
            def body():
                xt_sb = xt_pool.tile([128, QCH * ncol], mybir.dt.float16)
                if xt_split:
                    for qj in range(QCH):
                        getattr(nc, xt_engine).dma_start(
                            xt_sb[:, qj * ncol:(qj + 1) * ncol],
                            xt_dram[:, qj * ncol:(qj + 1) * ncol])
                else:
                    getattr(nc, xt_engine).dma_start(xt_sb[:], xt_dram[:])
                ps = [ps_pool.tile([128, ncol], mybir.dt.float32, tag="ps",
                                   name=f"ps{qi}")
                      for qi in range(QCH)]
                if not do_ops_dma or mm_src == "m0":
                    m0 = m0_pool.tile([128, gsz], mdt, tag="m0", bufs=1)
                    nc.sync.dma_start(m0[:], ops_dram[0])

                # progressive act/out: emit ReLU + out DMA for a column span
                # once the last slot covering it has been multiplied, instead
                # of serially after the whole stream.
                span_end = {}           # slot s -> (c0, c1) to flush after s
                o_prog = None
                if prog_out and do_act:
                    o_prog = o_pool.tile([128, QCH * ncol], odt, tag="o",
                                         bufs=1)
                    bounds = [nm * (j + 1) // prog_out - 1
                              for j in range(prog_out)]
                    c_prev = 0
                    for s_e in bounds:
                        c_hi = offs[s_e] + maxc[s_e]
                        span_end[s_e] = (c_prev, c_hi)
                        c_prev = c_hi

                def flush_span(c0, c1):
                    for qi in range(QCH):
                        dst = o_prog[:, qi * ncol + c0:qi * ncol + c1]
                        src = ps[qi][:, c0:c1]
                        if relu_engine == "vector":
                            nc.vector.tensor_scalar_max(dst, src, 0.0)
                        else:
                            nc.scalar.activation(
                                dst, src, mybir.ActivationFunctionType.Relu)
                        if do_out:
                            getattr(nc, out_engine).dma_start(
                                out_dram[:, qi * ncol + c0:qi * ncol + c1],
                                dst)

                for g in range(ng):
                    if do_ops_dma:
                        m = ops_pool.tile([128, gsz], mdt, tag="m")
                        if dma_split:
                            h = gsz // 2
                            nc.sync.dma_start(m[:, :h], ops_dram[g][:, :h])
                            nc.scalar.dma_start(m[:, h:], ops_dram[g][:, h:])
                        elif ops_engine is not None:
                            getattr(nc, ops_engine).dma_start(
                                m[:], ops_dram[g])
                        else:
                            issuer = nc.sync \
                                if g % sync_frac[1] < sync_frac[0] \
                                else nc.scalar
                            issuer.dma_start(m[:], ops_dram[g])
                        if mm_src == "m0":
                            m = m0
                    else:
                        m = m0
                    for t in range(pair):
                        s = g * pair + t
                        if s >= nm:
                            break
                        cw = maxc[s]
                        if not do_mm or s % mm_every:
                            continue
                        order = [(qi, qj) for qi in range(QCH)
                                 for qj in range(QCH)] \
                            if mm_order == "qi" else \
                            [(qi, qj) for qj in range(QCH)
                             for qi in range(QCH)]
                        for qi, qj in order:
                            ck = (((t * QCH + qj) * QCH + qi)
                                  % (gsz // 128)) * 128
                            lhsT = m[:, ck:ck + 128]
                            for h in range(mm_split):
                                a0 = offs[s] + cw * h // mm_split
                                a1 = offs[s] + cw * (h + 1) // mm_split
                                if a1 == a0:
                                    continue
                                rhs = xt_sb[:, qj * ncol + a0:
                                            qj * ncol + a1]
                                nc.tensor.matmul(
                                    ps[qi][:, a0:a1],
                                    lhsT, rhs, start=(qj == 0),
                                    stop=(qj == QCH - 1))
                if not do_act:
                    return
                o = o_pool.tile([128, QCH * ncol], odt, tag="o")
                for qi in range(QCH):
                    for h in range(act_split):
                        c0 = ncol * h // act_split
                        c1 = ncol * (h + 1) // act_split
                        dst = o[:, qi * ncol + c0:qi * ncol + c1]
                        src = ps[qi][:, c0:c1]
                        if relu_engine == "vector":
                            nc.vector.tensor_scalar_max(dst, src, 0.0)
                        else:
                            nc.scalar.activation(
                                dst, src, mybir.ActivationFunctionType.Relu)
                    if do_out and out_per_qi:
                        getattr(nc, out_engine).dma_start(
                            out_dram[:, qi * ncol:(qi + 1) * ncol],
                            o[:, qi * ncol:(qi + 1) * ncol])
                if do_out and not out_per_qi:
                    getattr(nc, out_engine).dma_start(out_dram[:], o[:])

            if reps == 1:
                body()
            else:
                with tc.For_i(0, reps, 1,
                              hint_engines=(mybir.EngineType.PE,),
                              staggered_reset=staggered):
                    body()

    nc.compile()
    return nc


NF = 17   # tail slots computed in non-flip orientation (hybrid)


def _build_raw(maxc, offs, ncol, reps=1, nb=63, do_ops_dma=True, do_mm=True,
               do_act=True, do_out=True, xt_chunks=1, use_regs=True,
               do_clear=False, ops_engine="sync", slot_stride=1,
               nf_probe=0, nf=None, pair=1):
    """Raw-bass (no Tile scheduling) flip-orientation kernel.

    Motivation (measured): Tile attaches a semaphore update to every matmul
    (1008/iter) and a full engine drain+reset between loop iterations; the
    per-instruction sem updates serialize at ~26ns each at the EVT_SEM
    register and the PE+DMA streams end up only ~40% overlapped (base 66.5us
    vs 46us DMA-alone line rate). This version hand-manages semaphores:
    one inc per ops-DMA (hardware, +16), one inc per GROUP on PE (last
    matmul's then_inc, sound because PE completions are pc-ordered), and a
    credit scheme whose wait targets are carried in per-engine registers so
    the hardware loop needs NO inter-iteration barrier: the ops-DMA ring
    spans iteration boundaries and the act/out tail of iteration i overlaps
    the DMA stream of iteration i+1.

    Engine program (per iteration):
      sync  : ng x [credit wait on pe_sem; dma_start ops -> ring (+16 dma_sem)]
      tensor: wait xt_sem; wait act_sem (PSUM free);
              ng x [wait dma_sem; 16 matmuls; last +1 pe_sem]
      scalar: wait pe_sem (xt free); dma xt (+16 xt_sem);
              4 x [wait act_sem; dma out qi (+16 out_sem)]
      vector: wait pe_sem (all groups); wait out_sem (o free);
              8 x relu chunk (+1 act_sem each)
    """
    nm = len(maxc)
    ng = nm
    if nf is None:
        nf = 0 if nf_probe else NF
    nf = min(nf, ng)
    # HEAD placement: slots 0..nf-1 are non-flip (measured ~0.9us faster
    # than tail placement); flip slots own columns [c_nf, ncol)
    assert all(maxc[s] <= 32 for s in range(nf))
    c_nf = offs[nf] if nf else 0                 # first flip column
    noffs = [0]                                  # out_nf row offsets
    for s in range(nf):
        noffs.append(noffs[-1] + maxc[s])
    n_out = 4 + nf                               # out DMAs per iteration
    n_act = 8 + nf                               # act_sem incs per iteration

    nc = bacc.Bacc("TRN2", target_bir_lowering=False, debug=False,
                   num_devices=N_CORES)
    gsz = QCH * QCH * 128
    # pair>1: `pair` matrices per DMA (bigger per-partition runs amortize
    # SDMA packet overhead); the last DMA is truncated so padding matrices
    # are never transferred. Forces whole-iteration ring (nb = ng).
    ng_dma = -(-ng // pair)
    gszp = pair * gsz
    if pair > 1:
        nb = ng
    ops_dram = nc.dram_tensor("ops_t", [ng_dma, 128, gszp],
                              mybir.dt.float8e3,
                              kind="ExternalInput").ap()
    xt_dram = nc.dram_tensor("xt", [128, QCH * ncol], mybir.dt.float16,
                             kind="ExternalInput").ap()
    out_dram = nc.dram_tensor("out", [128, QCH * ncol], mybir.dt.float16,
                              kind="ExternalOutput").ap()
    out_nf_dram = nc.dram_tensor(
        "out_nf", [max(1, noffs[-1]), D], mybir.dt.float16,
        kind="ExternalOutput").ap()

    xt_sb = nc.alloc_sbuf_tensor("xt_sb", [128, QCH * ncol],
                                 mybir.dt.float16).ap()
    o_sb = nc.alloc_sbuf_tensor("o_sb", [128, QCH * ncol],
                                mybir.dt.float16).ap()
    o_nf = nc.alloc_sbuf_tensor("o_nf", [128, 4 * D], mybir.dt.float16).ap()
    mring = nc.alloc_sbuf_tensor("mring", [128, ng_dma * gszp],
                                 mybir.dt.float8e3).ap()
    ps = [nc.alloc_psum_tensor(f"ps{qi}", [128, ncol], mybir.dt.float32).ap()
          for qi in range(QCH)]
    # 4 PSUM banks for the non-flip tail, cycled with reuse distance 4
    # (no tile_position partition packing: that path crashes the exec unit)
    ps_nf = [nc.alloc_psum_tensor(f"ps_nf{b}", [128, D],
                                  mybir.dt.float32).ap()
             for b in range(4)] if (nf or nf_probe) else None

    # dma completion sems: one LANE per ring slot. A single shared sem is
    # unsound with >1 DMA in flight: the 16 per-DMA increments come from 16
    # independent SDMA engines, so increments of DMA k+1 can stand in for
    # laggards of DMA k and a threshold wait passes before k fully lands
    # (observed: corruption starting exactly where PE catches the stream).
    # With lane=slot and ring credits bounding in-flight <= nb, each lane
    # has at most one DMA outstanding -> its count is unambiguous.
    # nb must divide ng so lane assignment is iteration-invariant.
    assert ng % nb == 0, (ng, nb)
    n_lanes = ng_dma if pair > 1 else nb
    dma_sems = [nc.alloc_semaphore(f"dma_sem{l}") for l in range(n_lanes)]
    xt_sem = nc.alloc_semaphore("xt_sem")
    pe_sem = nc.alloc_semaphore("pe_sem")
    act_sem = nc.alloc_semaphore("act_sem")
    out_sem = nc.alloc_semaphore("out_sem")
    sems = dma_sems + [xt_sem, pe_sem, act_sem, out_sem]

    # NB: do NOT clear sems at kernel start — both a gpsimd sem_clear +
    # _nrt_pseudo_barrier and a sem_clear + all_engine_barrier preamble
    # measurably RACE the engine streams here (verified: rel err jumps to
    # ~0.3-0.8). Instead sems are zeroed at kernel END (below), and the
    # program assumes zeroed sems at entry.
    if do_clear:
        for s in sems:
            nc.gpsimd.sem_clear(s)
        nc._nrt_pseudo_barrier()

    # ring credits: pre-seed nb so the first nb ops DMAs don't wait
    nc.tensor.sem_inc(pe_sem, nb)

    class _Ctr:
        """Cumulative wait target: a per-engine register, or (debug,
        reps==1 only) a compile-time constant."""
        def __init__(self, eng, name, init):
            self.eng, self.val = eng, init
            if use_regs:
                self.reg = eng.alloc_register(name)
                eng.reg_mov(self.reg, init)
        def add(self, d):
            self.val += d
            if use_regs:
                self.eng.reg_add(self.reg, self.reg, d)
        def wait(self, sem):
            self.eng.wait_ge(sem, self.reg if use_regs else self.val)
        def bump_to(self, target):
            # target sequences are iteration-invariant in their deltas,
            # so the emitted reg_adds replay correctly every loop pass
            assert target >= self.val, (target, self.val)
            if target > self.val:
                self.add(target - self.val)

    if not use_regs:
        assert reps == 1
    r_credit = _Ctr(nc.sync, "r_credit", 0)
    if nb == ng or pair > 1:
        # one DMA per lane per iteration -> every lane's target is the
        # same 16*(iter+1): share a single counter, bumped once per iter.
        r_dma_iter = _Ctr(nc.tensor, "r_dma", 16)
        r_dma = None
    else:
        r_dma = [_Ctr(nc.tensor, f"r_dma{l}", 0) for l in range(nb)]
    r_xt = _Ctr(nc.tensor, "r_xt", 16)
    r_actpe = _Ctr(nc.tensor, "r_actpe", 0)
    r_xtfree = _Ctr(nc.scalar, "r_xtfree", nb)
    r_act = _Ctr(nc.scalar, "r_act", 0)
    r_pe = _Ctr(nc.vector, "r_pe", nb)
    r_out = _Ctr(nc.vector, "r_out", 0)

    act_chunks = []   # (qi, c0, c1) in DVE issue order, 2 per qi; flip
    wf = ncol - c_nf    # columns only — nf head columns go via out_nf
    for qi in range(QCH):
        for h in range(2):
            act_chunks.append((qi, c_nf + wf * h // 2,
                               c_nf + wf * (h + 1) // 2))

    assert np.gcd(slot_stride, nb) == 1
    slot_of = lambda s: (s % nb) * slot_stride % nb

    with nc.Fori(0, reps, 1):
        # --- sync: ops stream into the ring ---
        if do_ops_dma and pair > 1:
            for d in range(ng_dma):
                # slot d's buffers hold groups [d*pair, min((d+1)*pair, ng))
                r_credit.bump_to(min(pair * (d + 1), ng))
                r_credit.wait(pe_sem)
                w = (min(pair * (d + 1), ng) - pair * d) * gsz
                nc.sync.dma_start(
                    mring[:, d * gszp:d * gszp + w], ops_dram[d][:, :w]
                ).then_inc(dma_sems[d], 16)
        elif do_ops_dma:
            for g in range(ng):
                r_credit.add(1)
                r_credit.wait(pe_sem)
                slot = slot_of(g)
                if ops_engine == "alt":
                    eng = nc.sync if g % 2 == 0 else nc.scalar
                else:
                    eng = getattr(nc, ops_engine)
                eng.dma_start(
                    mring[:, slot * gsz:(slot + 1) * gsz], ops_dram[g]
                ).then_inc(dma_sems[slot], 16)

        # --- scalar: xt in, outs out ---
        r_xtfree.wait(pe_sem)
        if xt_chunks == 1:
            nc.scalar.dma_start(xt_sb, xt_dram).then_inc(xt_sem, 16)
        else:
            w = QCH * ncol
            for c in range(xt_chunks):
                a, b = w * c // xt_chunks, w * (c + 1) // xt_chunks
                nc.scalar.dma_start(xt_sb[:, a:b], xt_dram[:, a:b]
                                    ).then_inc(xt_sem, 16)
        if do_out and do_act:
            for k in range(nf):
                u = k % 4
                cw = maxc[k]
                r_act.bump_to(k + 1)   # act inc of nf relu k (iter-0 vals)
                r_act.wait(act_sem)
                nc.scalar.dma_start(
                    out_nf_dram[noffs[k]:noffs[k] + cw, :],
                    o_nf[0:cw, u * D:(u + 1) * D]
                ).then_inc(out_sem, 16)
            for qi in range(QCH):
                r_act.bump_to(nf + 2 * (qi + 1))  # flip chunks come last
                r_act.wait(act_sem)
                nc.scalar.dma_start(
                    out_dram[:, qi * ncol + c_nf:(qi + 1) * ncol],
                    o_sb[:, qi * ncol + c_nf:(qi + 1) * ncol]
                ).then_inc(out_sem, 16)
        r_xtfree.add(ng)

        # --- tensor: the matmul stream ---
        r_xt.wait(xt_sem)
        r_actpe.wait(act_sem)
        for s in range(ng):
            if do_mm:
                if pair > 1:
                    d, t = divmod(s, pair)
                    if do_ops_dma and t == 0:
                        r_dma_iter.wait(dma_sems[d])
                    mbase = d * gszp + t * gsz
                else:
                    lane = slot_of(s)
                    if nb != ng:
                        r_dma[lane].add(16)
                    if do_ops_dma:
                        (r_dma_iter if nb == ng else r_dma[lane]).wait(
                            dma_sems[lane])
                    mbase = lane * gsz
                slot = None
                cw = maxc[s]
                last = None
                k = s if s < nf else -1
                if not nf:
                    k = -1
                if nf_probe < 0 and (s % 4 == 1 if nf_probe == -1
                                     else s < -nf_probe):
                    # TIMING PROBE ONLY (wrong math): non-flip-shaped slots
                    # placed mid-stream (-1: interleaved) or at the HEAD
                    # (-n: first n slots)
                    for qj in range(QCH):
                        lhsT = xt_sb[:, qj * ncol + offs[s]:
                                     qj * ncol + offs[s] + cw]
                        rhs = mring[:, mbase + qj * (QCH * 128):
                                    mbase + (qj + 1) * (QCH * 128)]
                        last = nc.tensor.matmul(
                            ps_nf[(s // 4) % 4][0:cw, :], lhsT, rhs,
                            start=(qj == 0), stop=(qj == QCH - 1))
                elif nf_probe > 0 and s >= ng - nf_probe:
                    # TIMING PROBE ONLY (wrong math): non-flip shape —
                    # 4 matmuls/matrix, x columns stationary, matrix moving.
                    for qj in range(QCH):
                        lhsT = xt_sb[:, qj * ncol + offs[s]:
                                     qj * ncol + offs[s] + cw]
                        rhs = mring[:, mbase + qj * (QCH * 128):
                                    mbase + (qj + 1) * (QCH * 128)]
                        last = nc.tensor.matmul(
                            ps_nf[0][0:cw, :], lhsT, rhs,
                            start=(qj == 0), stop=(qj == QCH - 1))
                elif k >= 0:
                    # non-flip: x columns stationary, matrix rows moving.
                    # PSUM: 4 dedicated banks, reused every 4 nf slots;
                    # a bank is freed by the eager DVE relu of slot k-4
                    # (act inc #(k-4+1) of this iteration: nf relus come
                    # FIRST in the act order under head placement).
                    b = k % 4
                    if k >= 4:
                        r_actpe.bump_to(k - 3)   # iter-0 target (k-4)+1
                        r_actpe.wait(act_sem)
                    for qj in range(QCH):
                        lhsT = xt_sb[:, qj * ncol + offs[s]:
                                     qj * ncol + offs[s] + cw]
                        rhs = mring[:, mbase + qj * D:
                                    mbase + (qj + 1) * D]
                        last = nc.tensor.matmul(
                            ps_nf[b][0:cw, :], lhsT, rhs,
                            start=(qj == 0), stop=(qj == QCH - 1))
                else:
                    for qi in range(QCH):
                        for qj in range(QCH):
                            ck = mbase + (qj * QCH + qi) * 128
                            lhsT = mring[:, ck:ck + 128]
                            rhs = xt_sb[:, qj * ncol + offs[s]:
                                        qj * ncol + offs[s] + cw]
                            last = nc.tensor.matmul(
                                ps[qi][:, offs[s]:offs[s] + cw], lhsT, rhs,
                                start=(qj == 0), stop=(qj == QCH - 1))
                last.then_inc(pe_sem, 1)
            else:
                nc.tensor.sem_inc(pe_sem, 1)
        r_xt.add(16)
        r_actpe.bump_to(n_act)
        if nb == ng:
            r_dma_iter.add(16)

        # --- vector: relu PSUM -> SBUF (nf head slots eagerly, then
        # the flip chunks once all flip slots are done) ---
        if do_act:
            r_out.wait(out_sem)          # prev iteration's outs landed
            for k in range(nf):
                u = k % 4
                cw = maxc[k]
                r_pe.bump_to(nb + k + 1)     # nf slot k done
                r_pe.wait(pe_sem)
                if k >= 4:
                    # o_nf column position reused: out DMA of slot k-4
                    # (out #(k-4+1) of this iteration) must have landed
                    r_out.bump_to(16 * (k - 3))
                    r_out.wait(out_sem)
                nc.vector.tensor_scalar_max(
                    o_nf[0:cw, u * D:(u + 1) * D],
                    ps_nf[u][0:cw, :], 0.0
                ).then_inc(act_sem, 1)
            r_pe.bump_to(nb + ng)        # all flip slots done
            r_pe.wait(pe_sem)
            for qi, c0, c1 in act_chunks:
                nc.vector.tensor_scalar_max(
                    o_sb[:, qi * ncol + c0:qi * ncol + c1],
                    ps[qi][:, c0:c1], 0.0
                ).then_inc(act_sem, 1)
            r_out.bump_to(16 * n_out)
        else:
            r_pe.add(ng)
            for _ in range(n_act):
                nc.vector.sem_inc(act_sem, 1)

    # quiesce: last iteration's out DMAs must have landed, then zero the
    # sems so a re-execution of this NEFF starts from clean state (waits
    # use absolute monotonic targets).
    if do_out and do_act:
        nc.scalar.wait_ge(out_sem, 64 * reps)
    nc.all_engine_barrier()
    for s in sems:
        nc.sync.sem_clear(s)
    nc.all_engine_barrier()
    nc.compile()
    return nc


def _route(attrs):
    """Group sample indices by attribute, chunk to <=128, snake-balance
    across cores. Returns per-core slot lists of (attr_id, idx_array),
    each list sorted by descending group size."""
    order = np.argsort(attrs, kind="stable")
    sorted_attrs = attrs[order]
    uniq, starts, counts = np.unique(sorted_attrs, return_index=True,
                                     return_counts=True)
    chunks = []
    for a, st, c in zip(uniq, starts, counts):
        idx = order[st:st + c]
        for o in range(0, c, 128):
            chunks.append((int(a), idx[o:o + 128]))
    chunks.sort(key=lambda t: -len(t[1]))
    per_core = [[] for _ in range(N_CORES)]
    for i, ch in enumerate(chunks):
        r, pos = divmod(i, N_CORES)
        k = pos if r % 2 == 0 else N_CORES - 1 - pos
        per_core[k].append(ch)
    return per_core


def _layout(per_core, align=1):
    """Per-slot-rank column capacity/offset shared by all cores.

    align: round capacities up so every slot's column offset is a multiple
    of `align` (align=2 makes f32 PSUM writes 8B-cacheline-aligned).
    """
    nm = max(1, max(len(s) for s in per_core))
    maxc = [1] * nm
    for slots in per_core:
        for s, (_, idx) in enumerate(slots):
            maxc[s] = max(maxc[s], len(idx))
    maxc = [-(-c // align) * align for c in maxc]
    offs = [0] * nm
    for s in range(1, nm):
        offs[s] = offs[s - 1] + maxc[s - 1]
    ncol = offs[-1] + maxc[-1]
    return nm, maxc, offs, ncol


def _prepare(attrs, objs, attr_ops, obj_emb, orient="flip", pair=None,
             align=None):
    """Route + build per-core device input maps."""
    if pair is None:
        pair = PAIR
    if align is None:
        align = ALIGN
    per_core = _route(attrs)
    nm, maxc, offs, ncol = _layout(per_core, align=align)
    nmp = -(-nm // pair) * pair

    rep = obj_emb[objs] * np.float32(1.0 / A_SCALE)  # [B, D], 1/s folded in
    ng = nmp // pair
    # raw hybrid: the last NF slots are computed non-flip on-device and
    # need the row-major (moving-operand) layout instead
    nf_eff = NF if (MODE == "raw" and orient == "flip") else 0
    in_maps = []
    for k in range(N_CORES):
        slots = per_core[k]
        ops_t = np.zeros((ng, 128, pair, QCH, QCH, 128), E3M4)
        r = np.zeros((ncol, D), np.float32)
        for s, (a, idx) in enumerate(slots):
            g, t = divmod(s, pair)
            at = np.clip(attr_ops[a].T * A_SCALE, -15.5, 15.5).astype(E3M4)
            if orient == "flip" and s >= nf_eff:
                # ops_t[g, p, t, qj, qi, i] = s*A[qi*128+i, qj*128+p]
                ops_t[g, :, t] = at.reshape(QCH, 128, QCH, 128).transpose(
                    1, 0, 2, 3)
            else:
                # ops_t[g, p, t, q, i] = s*A[i, q*128+p]
                ops_t[g, :, t] = at.reshape(QCH, 128, D).transpose(
                    1, 0, 2).reshape(128, QCH, QCH, 128)
            r[offs[s]:offs[s] + len(idx)] = rep[idx]
        # xt[p, q*ncol + c] = r[c, q*128 + p]
        xt = np.ascontiguousarray(r.reshape(ncol, QCH, 128).transpose(
            2, 1, 0).astype(np.float16)).reshape(128, -1)
        in_maps.append({"ops_t": ops_t.reshape(ng, 128, pair * QCH * D),
                        "xt": xt})
    return per_core, (nm, tuple(maxc), tuple(offs), ncol), in_maps


ORIENT = "flip"
MODE = "raw"      # "raw" (hand-managed sems) or "tile"


def _builder(reps=1, **kw):
    if MODE == "raw":
        def build(maxc, offs, ncol, **kw2):
            return _build_raw(list(maxc), list(offs), ncol, reps=reps,
                              **{**kw, **kw2})
    else:
        def build(maxc, offs, ncol, **kw2):
            b = _build_nc_flip if ORIENT == "flip" else _build_nc
            return b(list(maxc), list(offs), ncol, reps=reps, pair=PAIR,
                     **{**kw, **kw2})
    return build


def build_timing(maxc, offs, ncol, reps):
    """test.py hook: build the looped variant for wall-delta timing."""
    return _builder(reps=reps)(maxc, offs, ncol)


def kernel(attrs, objs, attr_ops, obj_emb):
    global LAST_RESULTS
    attrs = np.asarray(attrs)
    objs = np.asarray(objs)
    attr_ops = np.asarray(attr_ops, dtype=np.float32)
    obj_emb = np.asarray(obj_emb, dtype=np.float32)
    B = attrs.shape[0]
    d = obj_emb.shape[1]
    assert d == D and attr_ops.shape[1:] == (D, D)

    per_core, (nm, maxc, offs, ncol), in_maps = _prepare(
        attrs, objs, attr_ops, obj_emb, orient=ORIENT)

    nc = _NC_CACHE.get((MODE, ORIENT, maxc))
    if nc is None:
        nc = _NC_CACHE[(MODE, ORIENT, maxc)] = _builder()(maxc, offs, ncol)

    res = run_bass_kernel_spmd(nc, in_maps, core_ids=list(range(N_CORES)),
                               trace=TRACE, trace_cores=TRACE_CORES)
    LAST_RESULTS = res

    nf_eff = NF if (MODE == "raw" and ORIENT == "flip") else 0
    noffs = [0]
    for s in range(nf_eff):
        noffs.append(noffs[-1] + maxc[s])
    out = np.zeros((B, d), np.float32)
    for k in range(N_CORES):
        out_k = res.results[k]["out"].astype(np.float32)
        if ORIENT == "flip":
            out_k = out_k.reshape(128, QCH, ncol).transpose(2, 1, 0).reshape(
                ncol, D)
        for s, (a, idx) in enumerate(per_core[k]):
            if s < nf_eff:
                out_nf = res.results[k]["out_nf"].astype(np.float32)
                out[idx] = out_nf[noffs[s]:noffs[s] + len(idx)]
            else:
                out[idx] = out_k[offs[s]:offs[s] + len(idx)]
    return out

